# revision 1
# baseline (speedup 1.0000x reference)
"""Trainium2 Bass kernel for nn_AttentionBlock (GroupNorm + 8-head attention
block on [8, 512, 32, 32], residual).

Sharding: pure data-parallel over batch B=8 across the 8 NeuronCores — one
batch element per core, weights replicated, zero collectives.

Per-core dataflow (one batch element, x as [C=512, HW=1024] f32):
  1. GroupNorm(32 groups of 16 channels): raw sums via DVE reduce + ACT square
     accum, cross-partition group combine + expand via tiny matmuls with
     host-provided selector constants, then h = a*x + d on ACT (bf16 out).
  2. proj_in: q, k [512, 1024] (out-channels on partitions) and vT [1024, 520]
     (pixels on partitions; per head 64 v-columns + a constant ones column)
     via bf16 matmuls against host-pre-transposed w_inT.
  3. Per head pair: eT = exp(scale * k_h^T q_h) — the two heads' K=64 logits
     matmuls run concurrently in the two row-halves of the PE array
     (tile_position auto from base partitions); exp on ACT (bf16 out).
     out2[65, 1024] = [vT_h | ones]^T @ eT accumulated over the 8 k-tiles:
     rows 0..63 = unnormalized attention out, row 64 = softmax denominator.
     Reciprocal of the denominator row, then a DRAM round-trip DMA broadcast
     (stride-0 partition dim) to 64 partitions; attn_h = out2*recip + b_v.
  4. proj_out + bias + residual: matmul against host-pre-transposed w_outT,
     then one fused scalar_tensor_tensor: (psum + b_out) + x -> out f32.
"""
import sys

sys.path.insert(0, "/opt/trn_rl_repo")

import numpy as np
import ml_dtypes

import concourse.bass as bass
import concourse.bacc as bacc
import concourse.tile as tile
from concourse import mybir
from concourse.bass_utils import run_bass_kernel_spmd

F32 = mybir.dt.float32
BF16 = mybir.dt.bfloat16
ADD = mybir.AluOpType.add
MULT = mybir.AluOpType.mult

B, C, H, W = 8, 512, 32, 32
HW = H * W       # 1024
NG = 32          # groups
GS = C // NG     # 16 channels per group
NH = 8           # heads
HD = 64          # head dim
HID = NH * HD    # 512
EPS = 1e-6
SCALE = 1.0 / float(np.sqrt(HD))  # 0.125
CT = C // 128    # 4 channel partition-tiles
PT = HW // 128   # 8 pixel partition-tiles
GN_INV = 1.0 / (GS * HW)          # 1/16384


def build_graph():
    nc = bacc.Bacc("TRN2", num_devices=8)

    x_ext = nc.declare_dram_parameter("x", [C, HW], F32, isOutput=False)
    w_inT_ext = nc.declare_dram_parameter("w_inT", [C, 3 * HID], BF16, isOutput=False)
    w_outT_ext = nc.declare_dram_parameter("w_outT", [HID, C], BF16, isOutput=False)
    b_in_ext = nc.declare_dram_parameter("b_in_pm", [128, 12], F32, isOutput=False)
    b_v_ext = nc.declare_dram_parameter("b_v_pm", [HD, NH], F32, isOutput=False)
    b_out_ext = nc.declare_dram_parameter("b_out_pm", [128, CT], F32, isOutput=False)
    gamma_ext = nc.declare_dram_parameter("gamma_pm", [128, CT], F32, isOutput=False)
    beta_ext = nc.declare_dram_parameter("beta_pm", [128, CT], F32, isOutput=False)
    sel_ext = nc.declare_dram_parameter("gn_sel", [128, CT, NG], F32, isOutput=False)
    selT_ext = nc.declare_dram_parameter("gn_selT", [NG, CT, 128], F32, isOutput=False)
    out_ext = nc.declare_dram_parameter("out", [C, HW], F32, isOutput=True)

    recip_dram = nc.dram_tensor("recip_scratch", [NH, HW], F32)

    with tile.TileContext(nc) as tc:
        with (
            tc.tile_pool(name="const", bufs=1) as const,
            tc.tile_pool(name="big", bufs=1) as big,
            tc.tile_pool(name="eT", bufs=1) as eTp,
            tc.tile_pool(name="small", bufs=2) as small,
        ):
            # ---------- loads ----------
            x_sb = [big.tile([128, HW], F32, tag=f"x{t}", name=f"x{t}") for t in range(CT)]
            for t in range(CT):
                nc.gpsimd.dma_start(out=x_sb[t], in_=x_ext[128 * t:128 * (t + 1), :])
            w_inT_sb = [big.tile([128, 3 * HID], BF16, tag=f"wi{t}", name=f"wi{t}") for t in range(CT)]
            for t in range(CT):
                nc.gpsimd.dma_start(out=w_inT_sb[t],
                                    in_=w_inT_ext[128 * t:128 * (t + 1), :])
            w_outT_sb = [big.tile([128, C], BF16, tag=f"wo{t}", name=f"wo{t}") for t in range(CT)]
            for t in range(CT):
                nc.gpsimd.dma_start(out=w_outT_sb[t],
                                    in_=w_outT_ext[128 * t:128 * (t + 1), :])
            b_in_sb = const.tile([128, 12], F32)
            nc.gpsimd.dma_start(out=b_in_sb, in_=b_in_ext[:, :])
            b_v_sb = const.tile([HD, NH], F32)
            nc.gpsimd.dma_start(out=b_v_sb, in_=b_v_ext[:, :])
            b_out_sb = const.tile([128, CT], F32)
            nc.gpsimd.dma_start(out=b_out_sb, in_=b_out_ext[:, :])
            gamma_sb = const.tile([128, CT], F32)
            nc.gpsimd.dma_start(out=gamma_sb, in_=gamma_ext[:, :])
            beta_sb = const.tile([128, CT], F32)
            nc.gpsimd.dma_start(out=beta_sb, in_=beta_ext[:, :])
            sel_sb = const.tile([128, CT, NG], F32)
            nc.gpsimd.dma_start(out=sel_sb, in_=sel_ext[:, :, :])
            selT_sb = const.tile([NG, CT, 128], F32)
            nc.gpsimd.dma_start(out=selT_sb, in_=selT_ext[:, :, :])

            # ---------- groupnorm ----------
            with tc.tile_pool(name="ps_gn", bufs=2, space="PSUM") as ps_gn:
                stats = [small.tile([128, 2], F32, tag=f"st{t}", bufs=1, name=f"st{t}")
                         for t in range(CT)]
                sq_scratch = small.tile([128, HW], F32, tag="sqs", bufs=1)
                for t in range(CT):
                    nc.vector.reduce_sum(stats[t][:, 0:1], x_sb[t][:, :],
                                         axis=mybir.AxisListType.X)
                    nc.scalar.activation(out=sq_scratch, in_=x_sb[t][:, :],
                                         func=mybir.ActivationFunctionType.Square,
                                         accum_out=stats[t][:, 1:2])
                gpsum = ps_gn.tile([NG, 2], F32, tag="gps")
                for t in range(CT):
                    nc.tensor.matmul(gpsum[:, :], lhsT=sel_sb[:, t, :],
                                     rhs=stats[t][:, :],
                                     start=(t == 0), stop=(t == CT - 1))
                # grp cols: 0 rstd, 1 mean*rstd, 2 mean, 3 E[x^2] (later scratch)
                grp = small.tile([NG, 4], F32, tag="grp", bufs=1)
                eps_sb = small.tile([NG, 1], F32, tag="eps_c", bufs=1)
                nc.vector.memset(eps_sb, float(EPS))
                nc.vector.tensor_scalar_mul(grp[:, 2:4], gpsum[:, 0:2], GN_INV)
                nc.vector.tensor_mul(grp[:, 0:1], grp[:, 2:3], grp[:, 2:3])  # mean^2
                nc.vector.tensor_sub(grp[:, 0:1], grp[:, 3:4], grp[:, 0:1])  # var
                nc.scalar.activation(out=grp[:, 0:1], in_=grp[:, 0:1],
                                     func=mybir.ActivationFunctionType.Sqrt,
                                     bias=eps_sb[:, :], scale=1.0)
                nc.vector.reciprocal(out=grp[:, 0:1], in_=grp[:, 0:1])  # rstd
                nc.vector.tensor_mul(grp[:, 1:2], grp[:, 2:3], grp[:, 0:1])
                ga = [small.tile([128, 1], F32, tag=f"ga{t}", bufs=1, name=f"ga{t}")
                      for t in range(CT)]
                gd = [small.tile([128, 1], F32, tag=f"gd{t}", bufs=1, name=f"gd{t}")
                      for t in range(CT)]
                for t in range(CT):
                    epsum = ps_gn.tile([128, 2], F32, tag="eps")
                    nc.tensor.matmul(epsum[:, :], lhsT=selT_sb[:, t, :],
                                     rhs=grp[:, 0:2], start=True, stop=True)
                    nc.vector.tensor_mul(ga[t][:, :], gamma_sb[:, t:t + 1],
                                         epsum[:, 0:1])
                    # d = beta - gamma * (mean*rstd)
                    nc.vector.tensor_mul(gd[t][:, :], gamma_sb[:, t:t + 1],
                                         epsum[:, 1:2])
                    nc.vector.tensor_sub(gd[t][:, :], beta_sb[:, t:t + 1],
                                         gd[t][:, :])
                h_sb = [big.tile([128, HW], BF16, tag=f"h{t}", name=f"h{t}") for t in range(CT)]
                for t in range(CT):
                    nc.scalar.activation(out=h_sb[t], in_=x_sb[t][:, :],
                                         func=mybir.ActivationFunctionType.Identity,
                                         bias=gd[t][:, :], scale=ga[t][:, :])

            # ---------- proj_in ----------
            q_sb = [big.tile([128, HW], BF16, tag=f"q{m}", name=f"q{m}") for m in range(4)]
            k_sb = [big.tile([128, HW], BF16, tag=f"k{m}", name=f"k{m}") for m in range(4)]
            vT_sb = [big.tile([128, NH, HD + 1], BF16, tag=f"vT{p}", name=f"vT{p}")
                     for p in range(PT)]
            with tc.tile_pool(name="ps_pin", bufs=4, space="PSUM") as ps_pin:
                for dest, off in ((q_sb, 0), (k_sb, HID)):
                    for m in range(4):
                        bcol = (off + 128 * m) // 128
                        for n in range(2):
                            pp = ps_pin.tile([128, 512], F32, tag="pp")
                            for t in range(CT):
                                nc.tensor.matmul(
                                    pp[:, :],
                                    lhsT=w_inT_sb[t][:, off + 128 * m:
                                                     off + 128 * (m + 1)],
                                    rhs=h_sb[t][:, 512 * n:512 * (n + 1)],
                                    start=(t == 0), stop=(t == CT - 1))
                            nc.vector.tensor_scalar(
                                out=dest[m][:, 512 * n:512 * (n + 1)], in0=pp[:, :],
                                scalar1=b_in_sb[:, bcol:bcol + 1], scalar2=None,
                                op0=ADD)
                for p in range(PT):
                    nc.vector.memset(vT_sb[p], 1.0)
                for p in range(PT):
                    pp = ps_pin.tile([128, 512], F32, tag="pp")
                    for t in range(CT):
                        nc.tensor.matmul(
                            pp[:, :],
                            lhsT=h_sb[t][:, 128 * p:128 * (p + 1)],
                            rhs=w_inT_sb[t][:, 2 * HID:3 * HID],
                            start=(t == 0), stop=(t == CT - 1))
                    nc.vector.tensor_copy(
                        out=vT_sb[p][:, :, 0:HD],
                        in_=pp[:, :].rearrange("a (nh c) -> a nh c", nh=NH))

            # ---------- attention ----------
            attn_sb = [big.tile([128, HW], BF16, tag=f"at{i}", name=f"at{i}") for i in range(4)]
            with (
                tc.tile_pool(name="ps_log", bufs=2, space="PSUM") as ps_log,
                tc.tile_pool(name="ps_o2", bufs=2, space="PSUM") as ps_o2,
            ):
                eT_all = {}

                def emit_logits_exp(hp):
                    eTs = []
                    for sub in range(2):
                        eTs.append([eTp.tile([128, HW], BF16, bufs=2,
                                             tag=f"eT{sub}_{p}",
                                             name=f"eT{hp}_{sub}_{p}")
                                    for p in range(PT)])
                    eT_all[hp] = eTs
                    for p in range(PT):
                        pls = []
                        for sub in range(2):
                            lo, hi = 64 * sub, 64 * (sub + 1)
                            pl = ps_log.tile([128, HW], F32, tag="plog",
                                             name=f"pl{hp}_{sub}_{p}")
                            for n in range(2):
                                nc.tensor.matmul(
                                    pl[:, 512 * n:512 * (n + 1)],
                                    lhsT=k_sb[hp][lo:hi, 128 * p:128 * (p + 1)],
                                    rhs=q_sb[hp][lo:hi, 512 * n:512 * (n + 1)],
                                    start=True, stop=True)
                            pls.append(pl)
                        for sub in range(2):
                            nc.scalar.activation(
                                out=eTs[sub][p], in_=pls[sub][:, :],
                                func=mybir.ActivationFunctionType.Exp,
                                scale=SCALE)

                def emit_out2_norm(hp):
                    eTs = eT_all.pop(hp)
                    for sub in range(2):
                        head = 2 * hp + sub
                        eT = eTs[sub]
                        po = ps_o2.tile([HD + 1, HW], F32, tag="po2",
                                        name=f"po{head}")
                        for p in range(PT):
                            for n in range(2):
                                nc.tensor.matmul(
                                    po[:, 512 * n:512 * (n + 1)],
                                    lhsT=vT_sb[p][:, head, :],
                                    rhs=eT[p][:, 512 * n:512 * (n + 1)],
                                    start=(p == 0), stop=(p == PT - 1))
                        # early-evict unnormalized out (frees the PSUM slot)
                        attn_u = small.tile([64, HW], BF16, tag="attnu",
                                            bufs=4, name=f"attnu{head}")
                        nc.vector.tensor_copy(out=attn_u, in_=po[0:HD, :])
                        # fast reciprocal of denom row (PSUM p64 -> SBUF p64)
                        rrow = small.tile([HD + 1, HW], F32, tag="rrow",
                                          name=f"rrow{head}")
                        nc.vector.reciprocal(
                            out=rrow[HD:HD + 1, :], in_=po[HD:HD + 1, :])
                        nc.sync.dma_start(out=recip_dram[head:head + 1, :],
                                          in_=rrow[HD:HD + 1, :])
                        rb = small.tile([64, HW], F32, tag="rb",
                                        name=f"rb{head}")
                        bcast_ap = bass.AP(
                            tensor=recip_dram[:, :].tensor,
                            offset=head * HW,
                            ap=[[0, 64], [1, HW]])
                        nc.sync.dma_start(out=rb, in_=bcast_ap)
                        tmp = small.tile([64, HW], BF16, tag="atmp",
                                         name=f"atmp{head}")
                        nc.vector.tensor_mul(tmp[:, :], attn_u[:, :], rb[:, :])
                        if sub == 0:
                            nc.vector.tensor_scalar(
                                out=attn_sb[hp][0:64, :], in0=tmp[:, :],
                                scalar1=b_v_sb[:, head:head + 1],
                                scalar2=None, op0=ADD)
                        else:
                            tmp2 = small.tile([64, HW], BF16, tag="atmp2",
                                              name=f"atmp2{head}")
                            nc.vector.tensor_scalar(
                                out=tmp2[:, :], in0=tmp[:, :],
                                scalar1=b_v_sb[:, head:head + 1],
                                scalar2=None, op0=ADD)
                            nc.sync.dma_start(out=attn_sb[hp][64:128, :],
                                              in_=tmp2)

                # software-pipelined: logits/exp of pair hp overlap
                # out2/normalize of pair hp-1
                for step in range(5):
                    if step < 4:
                        emit_logits_exp(step)
                    if step >= 1:
                        emit_out2_norm(step - 1)

            # ---------- proj_out + bias + residual ----------
            with tc.tile_pool(name="ps_pout", bufs=4, space="PSUM") as ps_pout:
                for m in range(4):
                    for n in range(2):
                        pp = ps_pout.tile([128, 512], F32, tag="pp")
                        for t in range(CT):
                            nc.tensor.matmul(
                                pp[:, :],
                                lhsT=w_outT_sb[t][:, 128 * m:128 * (m + 1)],
                                rhs=attn_sb[t][:, 512 * n:512 * (n + 1)],
                                start=(t == 0), stop=(t == CT - 1))
                        o_sb = small.tile([128, 512], F32, tag="osb")
                        nc.vector.scalar_tensor_tensor(
                            out=o_sb, in0=pp[:, :], scalar=b_out_sb[:, m:m + 1],
                            in1=x_sb[m][:, 512 * n:512 * (n + 1)],
                            op0=ADD, op1=ADD)
                        nc.sync.dma_start(
                            out=out_ext[128 * m:128 * (m + 1),
                                        512 * n:512 * (n + 1)],
                            in_=o_sb)
    return nc


def _install_ntff_hook():
    """The agent image's antenv lacks axon_hooks; synthesize it so
    run_bass_kernel_spmd(trace=True) can reach the NTFF profiler."""
    import types
    if "antenv.axon_hooks" in sys.modules:
        return
    mod = types.ModuleType("antenv.axon_hooks")
    mod._hook = None

    def set_axon_ntff_profile_hook(hook):
        mod._hook = hook

    def get_axon_ntff_profile_hook():
        return mod._hook

    mod.set_axon_ntff_profile_hook = set_axon_ntff_profile_hook
    mod.get_axon_ntff_profile_hook = get_axon_ntff_profile_hook
    sys.modules["antenv.axon_hooks"] = mod
    try:
        from trn_agent_boot.trn_boot import _ntff_profile_via_ctypes
        hook = _ntff_profile_via_ctypes("/opt/axon/libaxon_pjrt.so")
        if hook is not None:
            set_axon_ntff_profile_hook(hook)
    except Exception as e:  # degrade to no tracing
        print("ntff hook setup failed:", e)


_COMPILED = None


def _get_compiled():
    global _COMPILED
    if _COMPILED is None:
        nc = build_graph()
        nc.compile()
        _COMPILED = nc
    return _COMPILED


def _make_consts():
    sel = np.zeros((128, CT, NG), dtype=np.float32)
    selT = np.zeros((NG, CT, 128), dtype=np.float32)
    for t in range(CT):
        for p in range(128):
            g = 8 * t + p // GS
            sel[p, t, g] = 1.0
            selT[g, t, p] = 1.0
    return sel, selT


def _pm(v, cols):
    """[cols*128] vector -> partition-major [128, cols]."""
    return np.ascontiguousarray(v.reshape(cols, 128).T)


def kernel(x, gamma, beta, w_in, b_in, w_out, b_out, _trace=False):
    x = np.asarray(x, dtype=np.float32)
    gamma = np.asarray(gamma, dtype=np.float32)
    beta = np.asarray(beta, dtype=np.float32)
    w_in = np.asarray(w_in, dtype=np.float32)
    b_in = np.asarray(b_in, dtype=np.float32)
    w_out = np.asarray(w_out, dtype=np.float32)
    b_out = np.asarray(b_out, dtype=np.float32)

    w_inT = np.ascontiguousarray(w_in.T).astype(ml_dtypes.bfloat16)
    w_outT = np.ascontiguousarray(w_out.T).astype(ml_dtypes.bfloat16)
    sel, selT = _make_consts()
    b_v = b_in[2 * HID:3 * HID]
    b_v_pm = np.ascontiguousarray(b_v.reshape(NH, HD).T)  # [64, 8]
    common = {
        "w_inT": w_inT,
        "w_outT": w_outT,
        "b_in_pm": _pm(b_in, 12),
        "b_v_pm": b_v_pm,
        "b_out_pm": _pm(b_out, CT),
        "gamma_pm": _pm(gamma, CT),
        "beta_pm": _pm(beta, CT),
        "gn_sel": sel,
        "gn_selT": selT,
    }
    in_maps = []
    for b in range(B):
        m = dict(common)
        m["x"] = np.ascontiguousarray(x[b].reshape(C, HW))
        in_maps.append(m)

    if _trace:
        _install_ntff_hook()
    nc = _get_compiled()
    res = run_bass_kernel_spmd(nc, in_maps, core_ids=list(range(B)),
                               trace=_trace)
    out = np.stack([np.asarray(res.results[b]["out"]).reshape(C, H, W)
                    for b in range(B)])
    if _trace:
        return out, res
    return out


if __name__ == "__main__":
    rng = np.random.default_rng(0)
    inputs = {
        "x": rng.standard_normal((B, C, H, W), dtype=np.float32),
        "gamma": np.ones(C, dtype=np.float32),
        "beta": np.zeros(C, dtype=np.float32),
        "w_in": (rng.standard_normal((3 * HID, C), dtype=np.float32)
                 / np.sqrt(C)),
        "b_in": np.zeros(3 * HID, dtype=np.float32),
        "w_out": (rng.standard_normal((C, HID), dtype=np.float32)
                  / np.sqrt(HID)),
        "b_out": np.zeros(C, dtype=np.float32),
    }
    out = kernel(**inputs)
    print("kernel ran, out shape", out.shape)



# revision 5
# speedup vs baseline: 1.1866x; 1.1866x over previous
"""Trainium2 Bass kernel for nn_AttentionBlock (GroupNorm + 8-head attention
block on [8, 512, 32, 32], residual).

Sharding: pure data-parallel over batch B=8 across the 8 NeuronCores — one
batch element per core, weights replicated, zero collectives.

v3 schedule, built around keeping the Activation engine dense on the 64
[128,1024] exps (the kernel is ACT-bound):
  - GroupNorm is per-channel-tile (each 16-channel group lives inside one
    128-channel tile, so no cross-tile combine): h[t] is ready ~3us after
    x[t] lands, and proj_in starts immediately.
  - proj_in (q/k per head-pair) is interleaved with attention: exp of pair 0
    starts right after qk0+logits0-p0 instead of after all of proj_in.
  - out2 (v @ eT) matmuls of pair hp ride along with the logits matmuls of
    pair hp+1; pair 3's out2 p-steps chase its own exps.
  - softmax denominators: po row 64 (from the vT ones-column) is evicted
    together with attn_u in one [65,1024] bf16 copy; the denom rows take a
    transposing DMA round trip to a [128,2,8] layout where one
    reciprocal_approx_fast costs ~0.2us (DVE time is free-size-based, so
    the single-partition RECIPROCAL of the baseline was 6.5us/head);
    reciprocals return via the inverse DMA and a stride-0 broadcast.
    (reciprocal_approx_fast reads garbage from PSUM — SBUF source only.)
  - b_v is folded into b_out on the host (softmax rows sum to 1, so
    out = w_out@attn_raw + (w_out@b_v + b_out) exactly).
  - GroupNorm's h = a*x+d runs on DVE (tensor_scalar mult+add) to keep ACT
    free for exp; only the vT ones-column is memset.
"""
import sys

sys.path.insert(0, "/opt/trn_rl_repo")

import numpy as np
import ml_dtypes

import concourse.bass as bass
import concourse.bacc as bacc
import concourse.tile as tile
from concourse import mybir
from concourse.bass_utils import run_bass_kernel_spmd

F32 = mybir.dt.float32
BF16 = mybir.dt.bfloat16
ADD = mybir.AluOpType.add
MULT = mybir.AluOpType.mult

B, C, H, W = 8, 512, 32, 32
HW = H * W       # 1024
NG = 32          # groups
GS = C // NG     # 16 channels per group
NH = 8           # heads
HD = 64          # head dim
HID = NH * HD    # 512
NP = NH // 2     # 4 head pairs
EPS = 1e-6
SCALE = 1.0 / float(np.sqrt(HD))  # 0.125
CT = C // 128    # 4 channel partition-tiles
PT = HW // 128   # 8 pixel partition-tiles
GPT = NG // CT   # 8 groups per channel-tile
GN_INV = 1.0 / (GS * HW)          # 1/16384


def build_graph():
    nc = bacc.Bacc("TRN2", num_devices=8)

    x_ext = nc.declare_dram_parameter("x", [C, HW], F32, isOutput=False)
    w_inT_ext = nc.declare_dram_parameter("w_inT", [C, 3 * HID], BF16, isOutput=False)
    w_outT_ext = nc.declare_dram_parameter("w_outT", [HID, C], BF16, isOutput=False)
    b_in_ext = nc.declare_dram_parameter("b_in_pm", [128, 8], F32, isOutput=False)
    b_out_ext = nc.declare_dram_parameter("b_out_pm", [128, CT], F32, isOutput=False)
    gamma_ext = nc.declare_dram_parameter("gamma_pm", [128, CT], F32, isOutput=False)
    beta_ext = nc.declare_dram_parameter("beta_pm", [128, CT], F32, isOutput=False)
    sel_ext = nc.declare_dram_parameter("gn_sel", [128, GPT], F32, isOutput=False)
    selT_ext = nc.declare_dram_parameter("gn_selT", [GPT, 128], F32, isOutput=False)
    out_ext = nc.declare_dram_parameter("out", [C, HW], F32, isOutput=True)

    den_dram = nc.dram_tensor("den_scratch", [NH, HW], BF16)
    rden_dram = nc.dram_tensor("rden_scratch", [NH, HW], BF16)

    with tile.TileContext(nc) as tc:
        with (
            tc.tile_pool(name="const", bufs=1) as const,
            tc.tile_pool(name="big", bufs=1) as big,
            tc.tile_pool(name="eT", bufs=1) as eTp,
            tc.tile_pool(name="small", bufs=2) as small,
        ):
            # ---------- loads: x first (GN critical path), then consts,
            # w_inT (needed at first qk matmul), w_outT last ----------
            x_sb = [big.tile([128, HW], F32, tag=f"x{t}", name=f"x{t}")
                    for t in range(CT)]
            for t in range(CT):
                for half in range(2):
                    nc.gpsimd.dma_start(
                        out=x_sb[t][:, 512 * half:512 * (half + 1)],
                        in_=x_ext[128 * t:128 * (t + 1),
                                  512 * half:512 * (half + 1)])
            gamma_sb = const.tile([128, CT], F32)
            nc.gpsimd.dma_start(out=gamma_sb, in_=gamma_ext[:, :])
            beta_sb = const.tile([128, CT], F32)
            nc.gpsimd.dma_start(out=beta_sb, in_=beta_ext[:, :])
            sel_sb = const.tile([128, GPT], F32)
            nc.gpsimd.dma_start(out=sel_sb, in_=sel_ext[:, :])
            selT_sb = const.tile([GPT, 128], F32)
            nc.gpsimd.dma_start(out=selT_sb, in_=selT_ext[:, :])
            b_in_sb = const.tile([128, 8], F32)
            nc.gpsimd.dma_start(out=b_in_sb, in_=b_in_ext[:, :])
            w_inT_sb = [big.tile([128, 3 * HID], BF16, tag=f"wi{t}", name=f"wi{t}")
                        for t in range(CT)]
            for t in range(CT):
                nc.gpsimd.dma_start(out=w_inT_sb[t],
                                    in_=w_inT_ext[128 * t:128 * (t + 1), :])
            w_outT_sb = [big.tile([128, C], BF16, tag=f"wo{t}", name=f"wo{t}")
                         for t in range(CT)]
            for t in range(CT):
                nc.gpsimd.dma_start(out=w_outT_sb[t],
                                    in_=w_outT_ext[128 * t:128 * (t + 1), :])
            b_out_sb = const.tile([128, CT], F32)
            nc.gpsimd.dma_start(out=b_out_sb, in_=b_out_ext[:, :])

            # ---------- groupnorm, fully per channel-tile ----------
            h_sb = [big.tile([128, HW], BF16, tag=f"h{t}", name=f"h{t}")
                    for t in range(CT)]
            with tc.tile_pool(name="ps_gn", bufs=2, space="PSUM") as ps_gn:
                eps_sb = small.tile([GPT, 1], F32, tag="eps_c", bufs=1)
                nc.vector.memset(eps_sb, float(EPS))
                sq_scratch = small.tile([128, HW], F32, tag="sqs", bufs=1)
                for t in range(CT):
                    st = small.tile([128, 2], F32, tag=f"st{t}", bufs=1,
                                    name=f"st{t}")
                    nc.vector.reduce_sum(st[:, 0:1], x_sb[t][:, :],
                                         axis=mybir.AxisListType.X)
                    nc.scalar.activation(out=sq_scratch, in_=x_sb[t][:, :],
                                         func=mybir.ActivationFunctionType.Square,
                                         accum_out=st[:, 1:2])
                    gpsum = ps_gn.tile([GPT, 2], F32, tag="gps")
                    nc.tensor.matmul(gpsum[:, :], lhsT=sel_sb[:, :],
                                     rhs=st[:, :], start=True, stop=True)
                    # grp cols: 0 rstd, 1 mean*rstd, 2 mean, 3 E[x^2]
                    grp = small.tile([GPT, 4], F32, tag="grp", bufs=2,
                                     name=f"grp{t}")
                    nc.vector.tensor_scalar_mul(grp[:, 2:4], gpsum[:, 0:2],
                                                GN_INV)
                    nc.vector.tensor_mul(grp[:, 0:1], grp[:, 2:3], grp[:, 2:3])
                    nc.vector.tensor_sub(grp[:, 0:1], grp[:, 3:4], grp[:, 0:1])
                    nc.scalar.activation(out=grp[:, 0:1], in_=grp[:, 0:1],
                                         func=mybir.ActivationFunctionType.Sqrt,
                                         bias=eps_sb[:, :], scale=1.0)
                    nc.vector.reciprocal(out=grp[:, 0:1], in_=grp[:, 0:1])
                    nc.vector.tensor_mul(grp[:, 1:2], grp[:, 2:3], grp[:, 0:1])
                    epsum = ps_gn.tile([128, 2], F32, tag="eps")
                    nc.tensor.matmul(epsum[:, :], lhsT=selT_sb[:, :],
                                     rhs=grp[:, 0:2], start=True, stop=True)
                    ga = small.tile([128, 1], F32, tag=f"ga{t}", bufs=1,
                                    name=f"ga{t}")
                    gd = small.tile([128, 1], F32, tag=f"gd{t}", bufs=1,
                                    name=f"gd{t}")
                    nc.vector.tensor_mul(ga[:, :], gamma_sb[:, t:t + 1],
                                         epsum[:, 0:1])
                    nc.vector.tensor_mul(gd[:, :], gamma_sb[:, t:t + 1],
                                         epsum[:, 1:2])
                    nc.vector.tensor_sub(gd[:, :], beta_sb[:, t:t + 1],
                                         gd[:, :])
                    # h = ga*x + gd on DVE (keeps ACT free for exp)
                    nc.vector.tensor_scalar(
                        out=h_sb[t], in0=x_sb[t][:, :],
                        scalar1=ga[:, :], scalar2=gd[:, :],
                        op0=MULT, op1=ADD)

            # ---------- fused proj_in + attention ----------
            q_sb = [big.tile([128, HW], BF16, tag=f"q{m}", name=f"q{m}")
                    for m in range(NP)]
            k_sb = [big.tile([128, HW], BF16, tag=f"k{m}", name=f"k{m}")
                    for m in range(NP)]
            vT_sb = [big.tile([128, NH, HD + 1], BF16, tag=f"vT{p}",
                              name=f"vT{p}") for p in range(PT)]
            attn_sb = [big.tile([128, HW], BF16, tag=f"at{i}", name=f"at{i}")
                       for i in range(NP)]
            po_tiles = {}    # hp -> [po_sub0, po_sub1]
            eT_all = {}      # hp -> [[eT tiles sub0], [sub1]]

            def emit_qk(hp):
                for dest, off, bc in ((q_sb, 0, hp), (k_sb, HID, 4 + hp)):
                    pp = pbig.tile([128, HW], F32, tag="pb",
                                   name=f"qk{off}_{hp}")
                    for n in range(2):
                        for t in range(CT):
                            nc.tensor.matmul(
                                pp[:, 512 * n:512 * (n + 1)],
                                lhsT=w_inT_sb[t][:, off + 128 * hp:
                                                 off + 128 * (hp + 1)],
                                rhs=h_sb[t][:, 512 * n:512 * (n + 1)],
                                start=(t == 0), stop=(t == CT - 1))
                    nc.vector.tensor_scalar(
                        out=dest[hp], in0=pp[:, :],
                        scalar1=b_in_sb[:, bc:bc + 1], scalar2=None, op0=ADD)

            def emit_v():
                for p in range(PT):
                    nc.vector.memset(vT_sb[p][:, :, HD:HD + 1], 1.0)
                for p in range(PT):
                    pp = pv.tile([128, 512], F32, tag="pv")
                    for t in range(CT):
                        nc.tensor.matmul(
                            pp[:, :],
                            lhsT=h_sb[t][:, 128 * p:128 * (p + 1)],
                            rhs=w_inT_sb[t][:, 2 * HID:3 * HID],
                            start=(t == 0), stop=(t == CT - 1))
                    nc.vector.tensor_copy(
                        out=vT_sb[p][:, :, 0:HD],
                        in_=pp[:, :].rearrange("a (nh c) -> a nh c", nh=NH))

            def out2_step(hp, p):
                """One accumulation p-step of pair hp's out2 (both heads)."""
                if p == 0:
                    po_tiles[hp] = [
                        pop.tile([HD + 1, HW], F32, tag="po",
                                 name=f"po{2 * hp + s}") for s in range(2)]
                eTs = eT_all[hp]
                for sub in range(2):
                    head = 2 * hp + sub
                    po_t = po_tiles[hp][sub]
                    for n in range(2):
                        nc.tensor.matmul(
                            po_t[:, 512 * n:512 * (n + 1)],
                            lhsT=vT_sb[p][:, head, :],
                            rhs=eTs[sub][p][:, 512 * n:512 * (n + 1)],
                            start=(p == 0), stop=(p == PT - 1))

            def emit_logits_exp(hp, out2_of=None):
                eTs = [[eTp.tile([128, HW], BF16, bufs=2, tag=f"eT{sub}_{p}",
                                 name=f"eT{hp}_{sub}_{p}") for p in range(PT)]
                       for sub in range(2)]
                eT_all[hp] = eTs
                for p in range(PT):
                    pls = []
                    for sub in range(2):
                        lo = 64 * sub
                        pl = pbig.tile([128, HW], F32, tag="pb",
                                       name=f"pl{hp}_{sub}_{p}")
                        for n in range(2):
                            nc.tensor.matmul(
                                pl[:, 512 * n:512 * (n + 1)],
                                lhsT=k_sb[hp][lo:lo + 64, 128 * p:128 * (p + 1)],
                                rhs=q_sb[hp][lo:lo + 64, 512 * n:512 * (n + 1)],
                                start=True, stop=True)
                        pls.append(pl)
                    for sub in range(2):
                        nc.scalar.activation(
                            out=eTs[sub][p], in_=pls[sub][:, :],
                            func=mybir.ActivationFunctionType.Exp,
                            scale=SCALE)
                    if out2_of is not None:
                        out2_step(out2_of, p)

            def finish_pair(hp):
                """Evict attn_u+denom, transpose-recip the denoms, normalize."""
                eT_all.pop(hp)
                pos = po_tiles.pop(hp)
                au65s = []
                for sub in range(2):
                    head = 2 * hp + sub
                    au65 = small.tile([HD + 1, HW], BF16, tag="attnu", bufs=4,
                                      name=f"attnu{head}")
                    nc.vector.tensor_copy(out=au65, in_=pos[sub][:, :])
                    nc.sync.dma_start(out=den_dram[head:head + 1, :],
                                      in_=au65[HD:HD + 1, :])
                    au65s.append(au65)
                # transpose-read the pair's denom rows: dT[p, s, c] =
                # den[2hp+s, 128c + p]; recip on 128 partitions x 16 elems.
                dTb = small.tile([128, 2, GPT], BF16, tag="dTb", bufs=2,
                                 name=f"dTb{hp}")
                tr_ap = bass.AP(tensor=den_dram[:, :].tensor,
                                offset=2 * hp * HW,
                                ap=[[1, 128], [HW, 2], [128, GPT]])
                nc.sync.dma_start(out=dTb, in_=tr_ap)
                dTf = small.tile([128, 2, GPT], F32, tag="dTf", bufs=2,
                                 name=f"dTf{hp}")
                nc.vector.tensor_copy(out=dTf, in_=dTb)
                rTf = small.tile([128, 2, GPT], F32, tag="rTf", bufs=2,
                                 name=f"rTf{hp}")
                nc.vector.reciprocal_approx_fast(out=rTf, in_=dTf)
                rTb = small.tile([128, 2, GPT], BF16, tag="rTb", bufs=2,
                                 name=f"rTb{hp}")
                nc.vector.tensor_copy(out=rTb, in_=rTf)
                wr_ap = bass.AP(tensor=rden_dram[:, :].tensor,
                                offset=2 * hp * HW,
                                ap=[[1, 128], [HW, 2], [128, GPT]])
                nc.sync.dma_start(out=wr_ap, in_=rTb)
                for sub in range(2):
                    head = 2 * hp + sub
                    rb = small.tile([HD, HW], BF16, tag="rb", bufs=4,
                                    name=f"rb{head}")
                    bcast_ap = bass.AP(
                        tensor=rden_dram[:, :].tensor,
                        offset=head * HW,
                        ap=[[0, HD], [1, HW]])
                    nc.sync.dma_start(out=rb, in_=bcast_ap)
                    if sub == 0:
                        nc.vector.tensor_mul(attn_sb[hp][0:HD, :],
                                             au65s[sub][0:HD, :], rb[:, :])
                    else:
                        tmp2 = small.tile([HD, HW], BF16, tag="atmp2",
                                          bufs=2, name=f"atmp2{head}")
                        nc.vector.tensor_mul(tmp2[:, :], au65s[sub][0:HD, :],
                                             rb[:, :])
                        nc.sync.dma_start(out=attn_sb[hp][HD:128, :],
                                          in_=tmp2)

            with tc.tile_pool(name="pbig", bufs=2, space="PSUM") as pbig:
                emit_qk(0)
                emit_logits_exp(0)
                emit_qk(1)
                with tc.tile_pool(name="pv", bufs=2, space="PSUM") as pv:
                    emit_v()
                with tc.tile_pool(name="po", bufs=2, space="PSUM") as pop:
                    emit_logits_exp(1, out2_of=0)
                    emit_qk(2)
                    finish_pair(0)
                    emit_logits_exp(2, out2_of=1)
                    emit_qk(3)
                    finish_pair(1)
                    emit_logits_exp(3, out2_of=2)
                    finish_pair(2)
                    for p in range(PT):
                        out2_step(3, p)
                    finish_pair(3)

            # ---------- proj_out + bias + residual ----------
            with tc.tile_pool(name="ps_pout", bufs=4, space="PSUM") as ps_pout:
                for m in range(CT):
                    for n in range(2):
                        pp = ps_pout.tile([128, 512], F32, tag="pp")
                        for t in range(CT):
                            nc.tensor.matmul(
                                pp[:, :],
                                lhsT=w_outT_sb[t][:, 128 * m:128 * (m + 1)],
                                rhs=attn_sb[t][:, 512 * n:512 * (n + 1)],
                                start=(t == 0), stop=(t == CT - 1))
                        o_sb = small.tile([128, 512], F32, tag="osb", bufs=4)
                        nc.vector.scalar_tensor_tensor(
                            out=o_sb, in0=pp[:, :],
                            scalar=b_out_sb[:, m:m + 1],
                            in1=x_sb[m][:, 512 * n:512 * (n + 1)],
                            op0=ADD, op1=ADD)
                        nc.sync.dma_start(
                            out=out_ext[128 * m:128 * (m + 1),
                                        512 * n:512 * (n + 1)],
                            in_=o_sb)
    return nc


def _install_ntff_hook():
    """The agent image's antenv lacks axon_hooks; synthesize it so
    run_bass_kernel_spmd(trace=True) can reach the NTFF profiler."""
    import types
    if "antenv.axon_hooks" in sys.modules:
        return
    mod = types.ModuleType("antenv.axon_hooks")
    mod._hook = None

    def set_axon_ntff_profile_hook(hook):
        mod._hook = hook

    def get_axon_ntff_profile_hook():
        return mod._hook

    mod.set_axon_ntff_profile_hook = set_axon_ntff_profile_hook
    mod.get_axon_ntff_profile_hook = get_axon_ntff_profile_hook
    sys.modules["antenv.axon_hooks"] = mod
    try:
        from trn_agent_boot.trn_boot import _ntff_profile_via_ctypes
        hook = _ntff_profile_via_ctypes("/opt/axon/libaxon_pjrt.so")
        if hook is not None:
            set_axon_ntff_profile_hook(hook)
    except Exception as e:  # degrade to no tracing
        print("ntff hook setup failed:", e)


_COMPILED = None


def _get_compiled():
    global _COMPILED
    if _COMPILED is None:
        nc = build_graph()
        nc.compile()
        _COMPILED = nc
    return _COMPILED


def _make_consts():
    # within any 128-channel tile, partition p belongs to local group p//16
    sel = np.zeros((128, GPT), dtype=np.float32)
    selT = np.zeros((GPT, 128), dtype=np.float32)
    for p in range(128):
        sel[p, p // GS] = 1.0
        selT[p // GS, p] = 1.0
    return sel, selT


def _pm(v, cols):
    """[cols*128] vector -> partition-major [128, cols]."""
    return np.ascontiguousarray(v.reshape(cols, 128).T)


def kernel(x, gamma, beta, w_in, b_in, w_out, b_out, _trace=False):
    x = np.asarray(x, dtype=np.float32)
    gamma = np.asarray(gamma, dtype=np.float32)
    beta = np.asarray(beta, dtype=np.float32)
    w_in = np.asarray(w_in, dtype=np.float32)
    b_in = np.asarray(b_in, dtype=np.float32)
    w_out = np.asarray(w_out, dtype=np.float32)
    b_out = np.asarray(b_out, dtype=np.float32)

    w_inT = np.ascontiguousarray(w_in.T).astype(ml_dtypes.bfloat16)
    w_outT = np.ascontiguousarray(w_out.T).astype(ml_dtypes.bfloat16)
    sel, selT = _make_consts()
    # fold v-bias through proj_out: softmax rows sum to 1, so the attention
    # output is attn_raw + b_v exactly; w_out @ b_v + b_out replaces b_out.
    b_v = b_in[2 * HID:3 * HID]
    b_out_eff = b_out + w_out.astype(np.float64) @ b_v.astype(np.float64)
    b_out_eff = b_out_eff.astype(np.float32)
    common = {
        "w_inT": w_inT,
        "w_outT": w_outT,
        "b_in_pm": _pm(b_in[0:2 * HID], 8),
        "b_out_pm": _pm(b_out_eff, CT),
        "gamma_pm": _pm(gamma, CT),
        "beta_pm": _pm(beta, CT),
        "gn_sel": sel,
        "gn_selT": selT,
    }
    in_maps = []
    for b in range(B):
        m = dict(common)
        m["x"] = np.ascontiguousarray(x[b].reshape(C, HW))
        in_maps.append(m)

    if _trace:
        _install_ntff_hook()
    nc = _get_compiled()
    res = run_bass_kernel_spmd(nc, in_maps, core_ids=list(range(B)),
                               trace=_trace)
    out = np.stack([np.asarray(res.results[b]["out"]).reshape(C, H, W)
                    for b in range(B)])
    if _trace:
        return out, res
    return out


if __name__ == "__main__":
    rng = np.random.default_rng(0)
    inputs = {
        "x": rng.standard_normal((B, C, H, W), dtype=np.float32),
        "gamma": np.ones(C, dtype=np.float32),
        "beta": np.zeros(C, dtype=np.float32),
        "w_in": (rng.standard_normal((3 * HID, C), dtype=np.float32)
                 / np.sqrt(C)),
        "b_in": np.zeros(3 * HID, dtype=np.float32),
        "w_out": (rng.standard_normal((C, HID), dtype=np.float32)
                  / np.sqrt(HID)),
        "b_out": np.zeros(C, dtype=np.float32),
    }
    out = kernel(**inputs)
    print("kernel ran, out shape", out.shape)


# revision 13
# speedup vs baseline: 1.4334x; 1.2080x over previous
"""Trainium2 Bass kernel for nn_AttentionBlock (GroupNorm + 8-head attention
block on [8, 512, 32, 32], residual).

Sharding: pure data-parallel over batch B=8 across the 8 NeuronCores — one
batch element per core, weights replicated, zero collectives.

v3 schedule, built around keeping the Activation engine dense on the 64
[128,1024] exps (the kernel is ACT-bound):
  - GroupNorm is per-channel-tile (each 16-channel group lives inside one
    128-channel tile, so no cross-tile combine): h[t] is ready ~3us after
    x[t] lands, and proj_in starts immediately.
  - proj_in (q/k per head-pair) is interleaved with attention: exp of pair 0
    starts right after qk0+logits0-p0 instead of after all of proj_in.
  - out2 (v @ eT) matmuls of pair hp ride along with the logits matmuls of
    pair hp+1; pair 3's out2 p-steps chase its own exps.
  - softmax denominators: po row 64 (from the vT ones-column) is evicted
    together with attn_u in one [65,1024] bf16 copy; the denom rows take a
    transposing DMA round trip to a [128,2,8] layout where one
    reciprocal_approx_fast costs ~0.2us (DVE time is free-size-based, so
    the single-partition RECIPROCAL of the baseline was 6.5us/head);
    reciprocals return via the inverse DMA and a stride-0 broadcast.
    (reciprocal_approx_fast reads garbage from PSUM — SBUF source only.)
  - b_v is folded into b_out on the host (softmax rows sum to 1, so
    out = w_out@attn_raw + (w_out@b_v + b_out) exactly).
  - GroupNorm's h = a*x+d runs on DVE (tensor_scalar mult+add) to keep ACT
    free for exp; only the vT ones-column is memset.
"""
import sys

sys.path.insert(0, "/opt/trn_rl_repo")

import numpy as np
import ml_dtypes

import concourse.bass as bass
import concourse.bacc as bacc
import concourse.tile as tile
from concourse import mybir
from concourse.bass_utils import run_bass_kernel_spmd

F32 = mybir.dt.float32
BF16 = mybir.dt.bfloat16
ADD = mybir.AluOpType.add
MULT = mybir.AluOpType.mult

B, C, H, W = 8, 512, 32, 32
HW = H * W       # 1024
NG = 32          # groups
GS = C // NG     # 16 channels per group
NH = 8           # heads
HD = 64          # head dim
HID = NH * HD    # 512
NP = NH // 2     # 4 head pairs
EPS = 1e-6
SCALE = 1.0 / float(np.sqrt(HD))  # 0.125
CT = C // 128    # 4 channel partition-tiles
PT = HW // 128   # 8 pixel partition-tiles
GPT = NG // CT   # 8 groups per channel-tile
GN_INV = 1.0 / (GS * HW)          # 1/16384


def build_graph():
    nc = bacc.Bacc("TRN2", num_devices=8)

    x_ext = nc.declare_dram_parameter("x", [C, HW], F32, isOutput=False)
    w_inT_ext = nc.declare_dram_parameter("w_inT", [C, 3 * HID], BF16, isOutput=False)
    w_outT_ext = nc.declare_dram_parameter("w_outT", [HID, C], BF16, isOutput=False)
    b_in_ext = nc.declare_dram_parameter("b_in_pm", [128, 8], F32, isOutput=False)
    b_out_ext = nc.declare_dram_parameter("b_out_pm", [128, CT], F32, isOutput=False)
    gamma_ext = nc.declare_dram_parameter("gamma_pm", [128, CT], F32, isOutput=False)
    beta_ext = nc.declare_dram_parameter("beta_pm", [128, CT], F32, isOutput=False)
    sel_ext = nc.declare_dram_parameter("gn_sel", [128, GPT], F32, isOutput=False)
    selT_ext = nc.declare_dram_parameter("gn_selT", [GPT, 128], F32, isOutput=False)
    out_ext = nc.declare_dram_parameter("out", [C, HW], F32, isOutput=True)

    rden_dram = nc.dram_tensor("rden_scratch", [NH, HW], F32)

    with tile.TileContext(nc) as tc:
        with (
            tc.tile_pool(name="const", bufs=1) as const,
            tc.tile_pool(name="big", bufs=1) as big,
            tc.tile_pool(name="eT", bufs=1) as eTp,
            tc.tile_pool(name="small", bufs=2) as small,
        ):
            # ---------- loads: x first (GN critical path), then consts,
            # w_inT (needed at first qk matmul), w_outT last ----------
            x_sb = [big.tile([128, HW], F32, tag=f"x{t}", name=f"x{t}")
                    for t in range(CT)]
            for t in range(CT):
                for qr in range(4):
                    nc.gpsimd.dma_start(
                        out=x_sb[t][:, 256 * qr:256 * (qr + 1)],
                        in_=x_ext[128 * t:128 * (t + 1),
                                  256 * qr:256 * (qr + 1)])
            gamma_sb = const.tile([128, CT], F32)
            nc.gpsimd.dma_start(out=gamma_sb, in_=gamma_ext[:, :])
            beta_sb = const.tile([128, CT], F32)
            nc.gpsimd.dma_start(out=beta_sb, in_=beta_ext[:, :])
            sel_sb = const.tile([128, GPT], F32)
            nc.gpsimd.dma_start(out=sel_sb, in_=sel_ext[:, :])
            selT_sb = const.tile([GPT, 128], F32)
            nc.gpsimd.dma_start(out=selT_sb, in_=selT_ext[:, :])
            b_in_sb = const.tile([128, 8], F32)
            nc.gpsimd.dma_start(out=b_in_sb, in_=b_in_ext[:, :])
            # w_inT column-sliced loads, in the order the schedule consumes
            # them: q/k cols for pairs 0-1, the v block, q/k pairs 2-3.
            w_inT_sb = [big.tile([128, 3 * HID], BF16, tag=f"wi{t}", name=f"wi{t}")
                        for t in range(CT)]

            def load_w_in_cols(lo, hi):
                for t in range(CT):
                    nc.gpsimd.dma_start(
                        out=w_inT_sb[t][:, lo:hi],
                        in_=w_inT_ext[128 * t:128 * (t + 1), lo:hi])

            for hp in (0, 1):
                load_w_in_cols(128 * hp, 128 * (hp + 1))              # q
                load_w_in_cols(HID + 128 * hp, HID + 128 * (hp + 1))  # k
            load_w_in_cols(2 * HID, 3 * HID)                          # v
            for hp in (2, 3):
                load_w_in_cols(128 * hp, 128 * (hp + 1))
                load_w_in_cols(HID + 128 * hp, HID + 128 * (hp + 1))
            w_outT_sb = [big.tile([128, C], BF16, tag=f"wo{t}", name=f"wo{t}")
                         for t in range(CT)]
            for t in range(CT):
                nc.gpsimd.dma_start(out=w_outT_sb[t],
                                    in_=w_outT_ext[128 * t:128 * (t + 1), :])
            b_out_sb = const.tile([128, CT], F32)
            nc.gpsimd.dma_start(out=b_out_sb, in_=b_out_ext[:, :])

            # ---------- groupnorm, fully per channel-tile ----------
            h_sb = [big.tile([128, HW], BF16, tag=f"h{t}", name=f"h{t}")
                    for t in range(CT)]
            with tc.tile_pool(name="ps_gn", bufs=2, space="PSUM") as ps_gn:
                eps_sb = small.tile([GPT, 1], F32, tag="eps_c", bufs=1)
                nc.vector.memset(eps_sb, float(EPS))
                sq_scratch = small.tile([128, HW], F32, tag="sqs", bufs=1)
                for t in range(CT):
                    st = small.tile([128, 2], F32, tag=f"st{t}", bufs=1,
                                    name=f"st{t}")
                    nc.vector.reduce_sum(st[:, 0:1], x_sb[t][:, :],
                                         axis=mybir.AxisListType.X)
                    nc.scalar.activation(out=sq_scratch, in_=x_sb[t][:, :],
                                         func=mybir.ActivationFunctionType.Square,
                                         accum_out=st[:, 1:2])
                    gpsum = ps_gn.tile([GPT, 2], F32, tag="gps")
                    nc.tensor.matmul(gpsum[:, :], lhsT=sel_sb[:, :],
                                     rhs=st[:, :], start=True, stop=True)
                    # grp cols: 0 rstd, 1 mean*rstd, 2 mean, 3 E[x^2]
                    grp = small.tile([GPT, 4], F32, tag="grp", bufs=2,
                                     name=f"grp{t}")
                    nc.vector.tensor_scalar_mul(grp[:, 2:4], gpsum[:, 0:2],
                                                GN_INV)
                    nc.vector.tensor_mul(grp[:, 0:1], grp[:, 2:3], grp[:, 2:3])
                    nc.vector.tensor_sub(grp[:, 0:1], grp[:, 3:4], grp[:, 0:1])
                    nc.scalar.activation(out=grp[:, 0:1], in_=grp[:, 0:1],
                                         func=mybir.ActivationFunctionType.Sqrt,
                                         bias=eps_sb[:, :], scale=1.0)
                    nc.vector.reciprocal(out=grp[:, 0:1], in_=grp[:, 0:1])
                    nc.vector.tensor_mul(grp[:, 1:2], grp[:, 2:3], grp[:, 0:1])
                    epsum = ps_gn.tile([128, 2], F32, tag="eps")
                    nc.tensor.matmul(epsum[:, :], lhsT=selT_sb[:, :],
                                     rhs=grp[:, 0:2], start=True, stop=True)
                    ga = small.tile([128, 1], F32, tag=f"ga{t}", bufs=1,
                                    name=f"ga{t}")
                    gd = small.tile([128, 1], F32, tag=f"gd{t}", bufs=1,
                                    name=f"gd{t}")
                    nc.vector.tensor_mul(ga[:, :], gamma_sb[:, t:t + 1],
                                         epsum[:, 0:1])
                    nc.vector.tensor_mul(gd[:, :], gamma_sb[:, t:t + 1],
                                         epsum[:, 1:2])
                    nc.vector.tensor_sub(gd[:, :], beta_sb[:, t:t + 1],
                                         gd[:, :])
                    # h = ga*x + gd on DVE (keeps ACT free for exp)
                    nc.vector.tensor_scalar(
                        out=h_sb[t], in0=x_sb[t][:, :],
                        scalar1=ga[:, :], scalar2=gd[:, :],
                        op0=MULT, op1=ADD)

            # ---------- fused proj_in + attention ----------
            q_sb = [big.tile([128, HW], BF16, tag=f"q{m}", name=f"q{m}")
                    for m in range(NP)]
            k_sb = [big.tile([128, HW], BF16, tag=f"k{m}", name=f"k{m}")
                    for m in range(NP)]
            vT_sb = [big.tile([128, NH, HD + 1], BF16, tag=f"vT{p}",
                              name=f"vT{p}") for p in range(PT)]
            attn_sb = [big.tile([128, HW], BF16, tag=f"at{i}", name=f"at{i}")
                       for i in range(NP)]
            po_tiles = {}    # hp -> [po_sub0, po_sub1]
            eT_all = {}      # hp -> [[eT tiles sub0], [sub1]]

            def emit_qk(hp):
                for dest, off, bc in ((q_sb, 0, hp), (k_sb, HID, 4 + hp)):
                    pp = pbig.tile([128, HW], F32, tag="pb",
                                   name=f"qk{off}_{hp}")
                    for n in range(2):
                        for t in range(CT):
                            nc.tensor.matmul(
                                pp[:, 512 * n:512 * (n + 1)],
                                lhsT=w_inT_sb[t][:, off + 128 * hp:
                                                 off + 128 * (hp + 1)],
                                rhs=h_sb[t][:, 512 * n:512 * (n + 1)],
                                start=(t == 0), stop=(t == CT - 1))
                    nc.vector.tensor_scalar(
                        out=dest[hp], in0=pp[:, :],
                        scalar1=b_in_sb[:, bc:bc + 1], scalar2=None, op0=ADD)

            def emit_v():
                for p in range(PT):
                    nc.vector.memset(vT_sb[p][:, :, HD:HD + 1], 1.0)
                for p in range(PT):
                    pp = pv.tile([128, 512], F32, tag="pv")
                    for t in range(CT):
                        nc.tensor.matmul(
                            pp[:, :],
                            lhsT=h_sb[t][:, 128 * p:128 * (p + 1)],
                            rhs=w_inT_sb[t][:, 2 * HID:3 * HID],
                            start=(t == 0), stop=(t == CT - 1))
                    nc.vector.tensor_copy(
                        out=vT_sb[p][:, :, 0:HD],
                        in_=pp[:, :].rearrange("a (nh c) -> a nh c", nh=NH))

            def out2_step(hp, p):
                """One accumulation p-step of pair hp's out2 (both heads)."""
                if p == 0:
                    po_tiles[hp] = [
                        pop.tile([HD + 1, HW], F32, tag="po",
                                 name=f"po{2 * hp + s}") for s in range(2)]
                eTs = eT_all[hp]
                for sub in range(2):
                    head = 2 * hp + sub
                    po_t = po_tiles[hp][sub]
                    for n in range(2):
                        nc.tensor.matmul(
                            po_t[:, 512 * n:512 * (n + 1)],
                            lhsT=vT_sb[p][:, head, :],
                            rhs=eTs[sub][p][:, 512 * n:512 * (n + 1)],
                            start=(p == 0), stop=(p == PT - 1))

            def emit_logits_exp(hp, out2_of=None):
                eTs = [[eTp.tile([128, HW], BF16, bufs=2, tag=f"eT{sub}_{p}",
                                 name=f"eT{hp}_{sub}_{p}") for p in range(PT)]
                       for sub in range(2)]
                eT_all[hp] = eTs
                for p in range(PT):
                    pls = []
                    for sub in range(2):
                        lo = 64 * sub
                        pl = pbig.tile([128, HW], F32, tag="pb",
                                       name=f"pl{hp}_{sub}_{p}")
                        for n in range(2):
                            nc.tensor.matmul(
                                pl[:, 512 * n:512 * (n + 1)],
                                lhsT=k_sb[hp][lo:lo + 64, 128 * p:128 * (p + 1)],
                                rhs=q_sb[hp][lo:lo + 64, 512 * n:512 * (n + 1)],
                                start=True, stop=True)
                        pls.append(pl)
                    for sub in range(2):
                        nc.scalar.activation(
                            out=eTs[sub][p], in_=pls[sub][:, :],
                            func=mybir.ActivationFunctionType.Exp,
                            scale=SCALE)
                    if out2_of is not None:
                        out2_step(out2_of, p)

            def finish_pair(hp):
                """Denominator reciprocal + normalize, reading po in place.

                reciprocal_approx_fast misreads PSUM, so the denom row takes
                one [1,1024] DVE hop to SBUF first. The multiply reads po
                directly (PSUM f32 x rb f32), so po stays live until here —
                safe, its pool buffer isn't needed again for ~a pair-window.
                """
                eT_all.pop(hp)
                pos = po_tiles.pop(hp)
                au_s = []
                den2 = small.tile([2, HW], F32, tag="den2", bufs=2,
                                  name=f"den2_{hp}")
                rr2 = small.tile([2, HW], F32, tag="rr2", bufs=2,
                                 name=f"rr2_{hp}")
                for sub in range(2):
                    head = 2 * hp + sub
                    au = small.tile([HD, HW], BF16, tag="attnu", bufs=4,
                                    name=f"attnu{head}")
                    nc.vector.tensor_copy(out=au, in_=pos[sub][0:HD, :])
                    au_s.append(au)
                    rrow = small.tile([HD + 1, HW], F32, tag="rrow",
                                      bufs=2, name=f"rrow{head}")
                    nc.vector.tensor_copy(out=rrow[HD:HD + 1, :],
                                          in_=pos[sub][HD:HD + 1, :])
                    # reciprocal_approx_fast is wrong at partition base != 0:
                    # hop the row down to partition `sub` via SBUF->SBUF DMA.
                    nc.sync.dma_start(out=den2[sub:sub + 1, :],
                                      in_=rrow[HD:HD + 1, :])
                nc.vector.reciprocal_approx_fast(out=rr2, in_=den2)
                nc.sync.dma_start(out=rden_dram[2 * hp:2 * hp + 2, :],
                                  in_=rr2)
                for sub in range(2):
                    head = 2 * hp + sub
                    rb = small.tile([HD, HW], F32, tag="rb", bufs=2,
                                    name=f"rb{head}")
                    bcast_ap = bass.AP(
                        tensor=rden_dram[:, :].tensor,
                        offset=head * HW,
                        ap=[[0, HD], [1, HW]])
                    nc.sync.dma_start(out=rb, in_=bcast_ap)
                    if sub == 0:
                        nc.vector.tensor_mul(attn_sb[hp][0:HD, :],
                                             au_s[sub][:, :], rb[:, :])
                    else:
                        tmp2 = small.tile([HD, HW], BF16, tag="atmp2",
                                          bufs=2, name=f"atmp2{head}")
                        nc.vector.tensor_mul(tmp2[:, :], au_s[sub][:, :],
                                             rb[:, :])
                        nc.sync.dma_start(out=attn_sb[hp][HD:128, :],
                                          in_=tmp2)

            with tc.tile_pool(name="pbig", bufs=2, space="PSUM") as pbig:
                emit_qk(0)
                emit_logits_exp(0)
                emit_qk(1)
                with tc.tile_pool(name="pv", bufs=2, space="PSUM") as pv:
                    emit_v()
                with tc.tile_pool(name="po", bufs=2, space="PSUM") as pop:
                    emit_logits_exp(1, out2_of=0)
                    emit_qk(2)
                    finish_pair(0)
                    emit_logits_exp(2, out2_of=1)
                    emit_qk(3)
                    finish_pair(1)
                    emit_logits_exp(3, out2_of=2)
                    finish_pair(2)
                    for p in range(PT):
                        out2_step(3, p)
                    finish_pair(3)

            # ---------- proj_out + bias + residual ----------
            with tc.tile_pool(name="ps_pout", bufs=4, space="PSUM") as ps_pout:
                for m in range(CT):
                    for n in range(2):
                        pp = ps_pout.tile([128, 512], F32, tag="pp")
                        for t in range(CT):
                            nc.tensor.matmul(
                                pp[:, :],
                                lhsT=w_outT_sb[t][:, 128 * m:128 * (m + 1)],
                                rhs=attn_sb[t][:, 512 * n:512 * (n + 1)],
                                start=(t == 0), stop=(t == CT - 1))
                        o_sb = small.tile([128, 512], F32, tag="osb", bufs=4)
                        nc.vector.scalar_tensor_tensor(
                            out=o_sb, in0=pp[:, :],
                            scalar=b_out_sb[:, m:m + 1],
                            in1=x_sb[m][:, 512 * n:512 * (n + 1)],
                            op0=ADD, op1=ADD)
                        nc.sync.dma_start(
                            out=out_ext[128 * m:128 * (m + 1),
                                        512 * n:512 * (n + 1)],
                            in_=o_sb)
    return nc


def _install_ntff_hook():
    """The agent image's antenv lacks axon_hooks; synthesize it so
    run_bass_kernel_spmd(trace=True) can reach the NTFF profiler."""
    import types
    if "antenv.axon_hooks" in sys.modules:
        return
    mod = types.ModuleType("antenv.axon_hooks")
    mod._hook = None

    def set_axon_ntff_profile_hook(hook):
        mod._hook = hook

    def get_axon_ntff_profile_hook():
        return mod._hook

    mod.set_axon_ntff_profile_hook = set_axon_ntff_profile_hook
    mod.get_axon_ntff_profile_hook = get_axon_ntff_profile_hook
    sys.modules["antenv.axon_hooks"] = mod
    try:
        from trn_agent_boot.trn_boot import _ntff_profile_via_ctypes
        hook = _ntff_profile_via_ctypes("/opt/axon/libaxon_pjrt.so")
        if hook is not None:
            set_axon_ntff_profile_hook(hook)
    except Exception as e:  # degrade to no tracing
        print("ntff hook setup failed:", e)


_COMPILED = None


def _get_compiled():
    global _COMPILED
    if _COMPILED is None:
        nc = build_graph()
        nc.compile()
        _COMPILED = nc
    return _COMPILED


def _make_consts():
    # within any 128-channel tile, partition p belongs to local group p//16
    sel = np.zeros((128, GPT), dtype=np.float32)
    selT = np.zeros((GPT, 128), dtype=np.float32)
    for p in range(128):
        sel[p, p // GS] = 1.0
        selT[p // GS, p] = 1.0
    return sel, selT


def _pm(v, cols):
    """[cols*128] vector -> partition-major [128, cols]."""
    return np.ascontiguousarray(v.reshape(cols, 128).T)


def kernel(x, gamma, beta, w_in, b_in, w_out, b_out, _trace=False):
    x = np.asarray(x, dtype=np.float32)
    gamma = np.asarray(gamma, dtype=np.float32)
    beta = np.asarray(beta, dtype=np.float32)
    w_in = np.asarray(w_in, dtype=np.float32)
    b_in = np.asarray(b_in, dtype=np.float32)
    w_out = np.asarray(w_out, dtype=np.float32)
    b_out = np.asarray(b_out, dtype=np.float32)

    w_inT = np.ascontiguousarray(w_in.T).astype(ml_dtypes.bfloat16)
    w_outT = np.ascontiguousarray(w_out.T).astype(ml_dtypes.bfloat16)
    sel, selT = _make_consts()
    # fold v-bias through proj_out: softmax rows sum to 1, so the attention
    # output is attn_raw + b_v exactly; w_out @ b_v + b_out replaces b_out.
    b_v = b_in[2 * HID:3 * HID]
    b_out_eff = b_out + w_out.astype(np.float64) @ b_v.astype(np.float64)
    b_out_eff = b_out_eff.astype(np.float32)
    common = {
        "w_inT": w_inT,
        "w_outT": w_outT,
        "b_in_pm": _pm(b_in[0:2 * HID], 8),
        "b_out_pm": _pm(b_out_eff, CT),
        "gamma_pm": _pm(gamma, CT),
        "beta_pm": _pm(beta, CT),
        "gn_sel": sel,
        "gn_selT": selT,
    }
    in_maps = []
    for b in range(B):
        m = dict(common)
        m["x"] = np.ascontiguousarray(x[b].reshape(C, HW))
        in_maps.append(m)

    if _trace:
        _install_ntff_hook()
    nc = _get_compiled()
    res = run_bass_kernel_spmd(nc, in_maps, core_ids=list(range(B)),
                               trace=_trace)
    out = np.stack([np.asarray(res.results[b]["out"]).reshape(C, H, W)
                    for b in range(B)])
    if _trace:
        return out, res
    return out


if __name__ == "__main__":
    rng = np.random.default_rng(0)
    inputs = {
        "x": rng.standard_normal((B, C, H, W), dtype=np.float32),
        "gamma": np.ones(C, dtype=np.float32),
        "beta": np.zeros(C, dtype=np.float32),
        "w_in": (rng.standard_normal((3 * HID, C), dtype=np.float32)
                 / np.sqrt(C)),
        "b_in": np.zeros(3 * HID, dtype=np.float32),
        "w_out": (rng.standard_normal((C, HID), dtype=np.float32)
                  / np.sqrt(HID)),
        "b_out": np.zeros(C, dtype=np.float32),
    }
    out = kernel(**inputs)
    print("kernel ran, out shape", out.shape)


# revision 14
# speedup vs baseline: 1.4990x; 1.0458x over previous
"""Trainium2 Bass kernel for nn_AttentionBlock (GroupNorm + 8-head attention
block on [8, 512, 32, 32], residual).

Sharding: pure data-parallel over batch B=8 across the 8 NeuronCores — one
batch element per core, weights replicated, zero collectives.

v3 schedule, built around keeping the Activation engine dense on the 64
[128,1024] exps (the kernel is ACT-bound):
  - GroupNorm is per-channel-tile (each 16-channel group lives inside one
    128-channel tile, so no cross-tile combine): h[t] is ready ~3us after
    x[t] lands, and proj_in starts immediately.
  - proj_in (q/k per head-pair) is interleaved with attention: exp of pair 0
    starts right after qk0+logits0-p0 instead of after all of proj_in.
  - out2 (v @ eT) matmuls of pair hp ride along with the logits matmuls of
    pair hp+1; pair 3's out2 p-steps chase its own exps.
  - softmax denominators: po row 64 (from the vT ones-column) is evicted
    together with attn_u in one [65,1024] bf16 copy; the denom rows take a
    transposing DMA round trip to a [128,2,8] layout where one
    reciprocal_approx_fast costs ~0.2us (DVE time is free-size-based, so
    the single-partition RECIPROCAL of the baseline was 6.5us/head);
    reciprocals return via the inverse DMA and a stride-0 broadcast.
    (reciprocal_approx_fast reads garbage from PSUM — SBUF source only.)
  - b_v is folded into b_out on the host (softmax rows sum to 1, so
    out = w_out@attn_raw + (w_out@b_v + b_out) exactly).
  - GroupNorm's h = a*x+d runs on DVE (tensor_scalar mult+add) to keep ACT
    free for exp; only the vT ones-column is memset.
"""
import sys

sys.path.insert(0, "/opt/trn_rl_repo")

import numpy as np
import ml_dtypes

import concourse.bass as bass
import concourse.bacc as bacc
import concourse.tile as tile
from concourse import mybir
from concourse.bass_utils import run_bass_kernel_spmd

F32 = mybir.dt.float32
BF16 = mybir.dt.bfloat16
ADD = mybir.AluOpType.add
MULT = mybir.AluOpType.mult

B, C, H, W = 8, 512, 32, 32
HW = H * W       # 1024
NG = 32          # groups
GS = C // NG     # 16 channels per group
NH = 8           # heads
HD = 64          # head dim
HID = NH * HD    # 512
NP = NH // 2     # 4 head pairs
EPS = 1e-6
SCALE = 1.0 / float(np.sqrt(HD))  # 0.125
CT = C // 128    # 4 channel partition-tiles
PT = HW // 128   # 8 pixel partition-tiles
GPT = NG // CT   # 8 groups per channel-tile
GN_INV = 1.0 / (GS * HW)          # 1/16384


def build_graph():
    nc = bacc.Bacc("TRN2", num_devices=8)

    x_ext = nc.declare_dram_parameter("x", [C, HW], BF16, isOutput=False)
    w_inT_ext = nc.declare_dram_parameter("w_inT", [C, 3 * HID], BF16, isOutput=False)
    w_outT_ext = nc.declare_dram_parameter("w_outT", [HID, C], BF16, isOutput=False)
    b_in_ext = nc.declare_dram_parameter("b_in_pm", [128, 8], F32, isOutput=False)
    b_out_ext = nc.declare_dram_parameter("b_out_pm", [128, CT], F32, isOutput=False)
    gamma_ext = nc.declare_dram_parameter("gamma_pm", [128, CT], F32, isOutput=False)
    beta_ext = nc.declare_dram_parameter("beta_pm", [128, CT], F32, isOutput=False)
    sel_ext = nc.declare_dram_parameter("gn_sel", [128, GPT], F32, isOutput=False)
    selT_ext = nc.declare_dram_parameter("gn_selT", [GPT, 128], F32, isOutput=False)
    out_ext = nc.declare_dram_parameter("out", [C, HW], F32, isOutput=True)

    rden_dram = nc.dram_tensor("rden_scratch", [NH, HW], F32)

    with tile.TileContext(nc) as tc:
        with (
            tc.tile_pool(name="const", bufs=1) as const,
            tc.tile_pool(name="big", bufs=1) as big,
            tc.tile_pool(name="eT", bufs=1) as eTp,
            tc.tile_pool(name="small", bufs=2) as small,
        ):
            # ---------- loads: x first (GN critical path), then consts,
            # w_inT (needed at first qk matmul), w_outT last ----------
            x_sb = [big.tile([128, HW], BF16, tag=f"x{t}", name=f"x{t}")
                    for t in range(CT)]
            for t in range(CT):
                for qr in range(8):
                    nc.gpsimd.dma_start(
                        out=x_sb[t][:, 128 * qr:128 * (qr + 1)],
                        in_=x_ext[128 * t:128 * (t + 1),
                                  128 * qr:128 * (qr + 1)])
            gamma_sb = const.tile([128, CT], F32)
            nc.gpsimd.dma_start(out=gamma_sb, in_=gamma_ext[:, :])
            beta_sb = const.tile([128, CT], F32)
            nc.gpsimd.dma_start(out=beta_sb, in_=beta_ext[:, :])
            sel_sb = const.tile([128, GPT], F32)
            nc.gpsimd.dma_start(out=sel_sb, in_=sel_ext[:, :])
            selT_sb = const.tile([GPT, 128], F32)
            nc.gpsimd.dma_start(out=selT_sb, in_=selT_ext[:, :])
            b_in_sb = const.tile([128, 8], F32)
            nc.gpsimd.dma_start(out=b_in_sb, in_=b_in_ext[:, :])
            # w_inT column-sliced loads, in the order the schedule consumes
            # them: q/k cols for pairs 0-1, the v block, q/k pairs 2-3.
            w_inT_sb = [big.tile([128, 3 * HID], BF16, tag=f"wi{t}", name=f"wi{t}")
                        for t in range(CT)]

            def load_w_in_cols(lo, hi):
                for t in range(CT):
                    nc.gpsimd.dma_start(
                        out=w_inT_sb[t][:, lo:hi],
                        in_=w_inT_ext[128 * t:128 * (t + 1), lo:hi])

            for hp in (0, 1):
                load_w_in_cols(128 * hp, 128 * (hp + 1))              # q
                load_w_in_cols(HID + 128 * hp, HID + 128 * (hp + 1))  # k
            load_w_in_cols(2 * HID, 3 * HID)                          # v
            for hp in (2, 3):
                load_w_in_cols(128 * hp, 128 * (hp + 1))
                load_w_in_cols(HID + 128 * hp, HID + 128 * (hp + 1))
            w_outT_sb = [big.tile([128, C], BF16, tag=f"wo{t}", name=f"wo{t}")
                         for t in range(CT)]
            for t in range(CT):
                nc.gpsimd.dma_start(out=w_outT_sb[t],
                                    in_=w_outT_ext[128 * t:128 * (t + 1), :])
            b_out_sb = const.tile([128, CT], F32)
            nc.gpsimd.dma_start(out=b_out_sb, in_=b_out_ext[:, :])

            # ---------- groupnorm (per channel-tile) + qk0, interleaved ----------
            h_sb = [big.tile([128, HW], BF16, tag=f"h{t}", name=f"h{t}")
                    for t in range(CT)]
            # ---------- fused proj_in + attention ----------
            q_sb = [big.tile([128, HW], BF16, tag=f"q{m}", name=f"q{m}")
                    for m in range(NP)]
            k_sb = [big.tile([128, HW], BF16, tag=f"k{m}", name=f"k{m}")
                    for m in range(NP)]
            vT_sb = [big.tile([128, NH, HD + 1], BF16, tag=f"vT{p}",
                              name=f"vT{p}") for p in range(PT)]
            attn_sb = [big.tile([128, HW], BF16, tag=f"at{i}", name=f"at{i}")
                       for i in range(NP)]
            po_tiles = {}    # hp -> [po_sub0, po_sub1]
            eT_all = {}      # hp -> [[eT tiles sub0], [sub1]]

            def qk_burst(hp, which):
                """All 8 accumulation matmuls + evict for q or k of pair hp,
                emitted as one slot so the pbig ring is never held open."""
                dest, off, bc = ((q_sb, 0, hp) if which == "q"
                                 else (k_sb, HID, 4 + hp))
                pp = pbig.tile([128, HW], F32, tag="pb",
                               name=f"{which}_acc{hp}")
                for n in range(2):
                    for t in range(CT):
                        nc.tensor.matmul(
                            pp[:, 512 * n:512 * (n + 1)],
                            lhsT=w_inT_sb[t][:, off + 128 * hp:
                                             off + 128 * (hp + 1)],
                            rhs=h_sb[t][:, 512 * n:512 * (n + 1)],
                            start=(t == 0), stop=(t == CT - 1))
                nc.vector.tensor_scalar(
                    out=dest[hp], in0=pp[:, :],
                    scalar1=b_in_sb[:, bc:bc + 1], scalar2=None, op0=ADD)

            def v_burst(j):
                """v tiles 2j, 2j+1 (pv pool, own PSUM banks)."""
                for p in (2 * j, 2 * j + 1):
                    pp = pv.tile([128, 512], F32, tag="pv")
                    for t in range(CT):
                        nc.tensor.matmul(
                            pp[:, :],
                            lhsT=h_sb[t][:, 128 * p:128 * (p + 1)],
                            rhs=w_inT_sb[t][:, 2 * HID:3 * HID],
                            start=(t == 0), stop=(t == CT - 1))
                    nc.vector.tensor_copy(
                        out=vT_sb[p][:, :, 0:HD],
                        in_=pp[:, :].rearrange("a (nh c) -> a nh c", nh=NH))

            def out2_step(hp, p):
                """One accumulation p-step of pair hp's out2 (both heads)."""
                if p == 0:
                    po_tiles[hp] = [
                        pop.tile([HD + 1, HW], F32, tag="po",
                                 name=f"po{2 * hp + s}") for s in range(2)]
                eTs = eT_all[hp]
                for sub in range(2):
                    head = 2 * hp + sub
                    po_t = po_tiles[hp][sub]
                    for n in range(2):
                        nc.tensor.matmul(
                            po_t[:, 512 * n:512 * (n + 1)],
                            lhsT=vT_sb[p][:, head, :],
                            rhs=eTs[sub][p][:, 512 * n:512 * (n + 1)],
                            start=(p == 0), stop=(p == PT - 1))

            def emit_logits_exp(hp, out2_of=None, fillers=None):
                eTs = [[eTp.tile([128, HW], BF16, bufs=2, tag=f"eT{sub}_{p}",
                                 name=f"eT{hp}_{sub}_{p}") for p in range(PT)]
                       for sub in range(2)]
                eT_all[hp] = eTs
                for p in range(PT):
                    pls = []
                    for sub in range(2):
                        lo = 64 * sub
                        pl = pbig.tile([128, HW], F32, tag="pb",
                                       name=f"pl{hp}_{sub}_{p}")
                        for n in range(2):
                            nc.tensor.matmul(
                                pl[:, 512 * n:512 * (n + 1)],
                                lhsT=k_sb[hp][lo:lo + 64, 128 * p:128 * (p + 1)],
                                rhs=q_sb[hp][lo:lo + 64, 512 * n:512 * (n + 1)],
                                start=True, stop=True)
                        pls.append(pl)
                    for sub in range(2):
                        nc.scalar.activation(
                            out=eTs[sub][p], in_=pls[sub][:, :],
                            func=mybir.ActivationFunctionType.Exp,
                            scale=SCALE)
                    if out2_of is not None:
                        out2_step(out2_of, p)
                    for f in (fillers or {}).get(p, []):
                        f()

            def finish_pair(hp):
                """Denominator reciprocal + normalize, reading po in place.

                reciprocal_approx_fast misreads PSUM, so the denom row takes
                one [1,1024] DVE hop to SBUF first. The multiply reads po
                directly (PSUM f32 x rb f32), so po stays live until here —
                safe, its pool buffer isn't needed again for ~a pair-window.
                """
                eT_all.pop(hp)
                pos = po_tiles.pop(hp)
                au_s = []
                den2 = small.tile([2, HW], F32, tag="den2", bufs=2,
                                  name=f"den2_{hp}")
                rr2 = small.tile([2, HW], F32, tag="rr2", bufs=2,
                                 name=f"rr2_{hp}")
                for sub in range(2):
                    head = 2 * hp + sub
                    au = small.tile([HD, HW], BF16, tag="attnu", bufs=4,
                                    name=f"attnu{head}")
                    nc.vector.tensor_copy(out=au, in_=pos[sub][0:HD, :])
                    au_s.append(au)
                    rrow = small.tile([HD + 1, HW], F32, tag="rrow",
                                      bufs=2, name=f"rrow{head}")
                    nc.vector.tensor_copy(out=rrow[HD:HD + 1, :],
                                          in_=pos[sub][HD:HD + 1, :])
                    # reciprocal_approx_fast is wrong at partition base != 0:
                    # hop the row down to partition `sub` via SBUF->SBUF DMA.
                    nc.sync.dma_start(out=den2[sub:sub + 1, :],
                                      in_=rrow[HD:HD + 1, :])
                nc.vector.reciprocal_approx_fast(out=rr2, in_=den2)
                nc.sync.dma_start(out=rden_dram[2 * hp:2 * hp + 2, :],
                                  in_=rr2)
                for sub in range(2):
                    head = 2 * hp + sub
                    rb = small.tile([HD, HW], F32, tag="rb", bufs=2,
                                    name=f"rb{head}")
                    bcast_ap = bass.AP(
                        tensor=rden_dram[:, :].tensor,
                        offset=head * HW,
                        ap=[[0, HD], [1, HW]])
                    nc.sync.dma_start(out=rb, in_=bcast_ap)
                    if sub == 0:
                        nc.vector.tensor_mul(attn_sb[hp][0:HD, :],
                                             au_s[sub][:, :], rb[:, :])
                    else:
                        tmp2 = small.tile([HD, HW], BF16, tag="atmp2",
                                          bufs=2, name=f"atmp2{head}")
                        nc.vector.tensor_mul(tmp2[:, :], au_s[sub][:, :],
                                             rb[:, :])
                        nc.sync.dma_start(out=attn_sb[hp][HD:128, :],
                                          in_=tmp2)

            with tc.tile_pool(name="pbig", bufs=2, space="PSUM") as pbig:
                # GN per tile + qk0 accumulation steps ride along so the
                # first exp fires as soon as x3 lands.
                ppq0 = pbig.tile([128, HW], F32, tag="pb", name="q_acc0")
                ppk0 = pbig.tile([128, HW], F32, tag="pb", name="k_acc0")
                with tc.tile_pool(name="ps_gn", bufs=2, space="PSUM") as ps_gn:
                    eps_sb = small.tile([GPT, 1], F32, tag="eps_c", bufs=1)
                    nc.vector.memset(eps_sb, float(EPS))
                    sq_scratch = small.tile([128, HW], F32, tag="sqs", bufs=1)
                    for t in range(CT):
                        st = small.tile([128, 2], F32, tag=f"st{t}", bufs=1,
                                        name=f"st{t}")
                        nc.vector.reduce_sum(st[:, 0:1], x_sb[t][:, :],
                                             axis=mybir.AxisListType.X)
                        # sum of squares on DVE: (x bypass) * x, accum free
                        nc.vector.scalar_tensor_tensor(
                            out=sq_scratch, in0=x_sb[t][:, :], scalar=1.0,
                            in1=x_sb[t][:, :],
                            op0=mybir.AluOpType.bypass, op1=MULT,
                            accum_out=st[:, 1:2])
                        gpsum = ps_gn.tile([GPT, 2], F32, tag="gps")
                        nc.tensor.matmul(gpsum[:, :], lhsT=sel_sb[:, :],
                                         rhs=st[:, :], start=True, stop=True)
                        # grp cols: 0 rstd, 1 mean*rstd, 2 mean, 3 E[x^2]
                        grp = small.tile([GPT, 4], F32, tag="grp", bufs=2,
                                         name=f"grp{t}")
                        nc.vector.tensor_scalar_mul(grp[:, 2:4],
                                                    gpsum[:, 0:2], GN_INV)
                        nc.vector.tensor_mul(grp[:, 0:1], grp[:, 2:3],
                                             grp[:, 2:3])
                        nc.vector.tensor_sub(grp[:, 0:1], grp[:, 3:4],
                                             grp[:, 0:1])
                        nc.scalar.activation(
                            out=grp[:, 0:1], in_=grp[:, 0:1],
                            func=mybir.ActivationFunctionType.Sqrt,
                            bias=eps_sb[:, :], scale=1.0)
                        nc.vector.reciprocal(out=grp[:, 0:1], in_=grp[:, 0:1])
                        nc.vector.tensor_mul(grp[:, 1:2], grp[:, 2:3],
                                             grp[:, 0:1])
                        epsum = ps_gn.tile([128, 2], F32, tag="eps")
                        nc.tensor.matmul(epsum[:, :], lhsT=selT_sb[:, :],
                                         rhs=grp[:, 0:2], start=True,
                                         stop=True)
                        ga = small.tile([128, 1], F32, tag=f"ga{t}", bufs=1,
                                        name=f"ga{t}")
                        gd = small.tile([128, 1], F32, tag=f"gd{t}", bufs=1,
                                        name=f"gd{t}")
                        nc.vector.tensor_mul(ga[:, :], gamma_sb[:, t:t + 1],
                                             epsum[:, 0:1])
                        nc.vector.tensor_mul(gd[:, :], gamma_sb[:, t:t + 1],
                                             epsum[:, 1:2])
                        nc.vector.tensor_sub(gd[:, :], beta_sb[:, t:t + 1],
                                             gd[:, :])
                        nc.vector.tensor_scalar(
                            out=h_sb[t], in0=x_sb[t][:, :],
                            scalar1=ga[:, :], scalar2=gd[:, :],
                            op0=MULT, op1=ADD)
                        for n in range(2):
                            nc.tensor.matmul(
                                ppq0[:, 512 * n:512 * (n + 1)],
                                lhsT=w_inT_sb[t][:, 0:128],
                                rhs=h_sb[t][:, 512 * n:512 * (n + 1)],
                                start=(t == 0), stop=(t == CT - 1))
                            nc.tensor.matmul(
                                ppk0[:, 512 * n:512 * (n + 1)],
                                lhsT=w_inT_sb[t][:, HID:HID + 128],
                                rhs=h_sb[t][:, 512 * n:512 * (n + 1)],
                                start=(t == 0), stop=(t == CT - 1))
                nc.vector.tensor_scalar(
                    out=q_sb[0], in0=ppq0[:, :],
                    scalar1=b_in_sb[:, 0:1], scalar2=None, op0=ADD)
                nc.vector.tensor_scalar(
                    out=k_sb[0], in0=ppk0[:, :],
                    scalar1=b_in_sb[:, 4:5], scalar2=None, op0=ADD)
                for p in range(PT):
                    nc.vector.memset(vT_sb[p][:, :, HD:HD + 1], 1.0)
                with tc.tile_pool(name="pv", bufs=2, space="PSUM") as pv:
                    emit_logits_exp(0, fillers={
                        1: [lambda: qk_burst(1, "q")],
                        3: [lambda: v_burst(0)],
                        4: [lambda: v_burst(1)],
                        5: [lambda: qk_burst(1, "k")],
                        6: [lambda: v_burst(2)],
                        7: [lambda: v_burst(3)],
                    })
                with tc.tile_pool(name="po", bufs=2, space="PSUM") as pop:
                    emit_logits_exp(1, out2_of=0, fillers={
                        2: [lambda: qk_burst(2, "q")],
                        5: [lambda: qk_burst(2, "k")],
                    })
                    finish_pair(0)
                    emit_logits_exp(2, out2_of=1, fillers={
                        2: [lambda: qk_burst(3, "q")],
                        5: [lambda: qk_burst(3, "k")],
                    })
                    finish_pair(1)
                    emit_logits_exp(3, out2_of=2)
                    finish_pair(2)
                    for p in range(PT):
                        out2_step(3, p)
                    finish_pair(3)

            # ---------- proj_out + bias + residual ----------
            with tc.tile_pool(name="ps_pout", bufs=4, space="PSUM") as ps_pout:
                for m in range(CT):
                    for n in range(2):
                        pp = ps_pout.tile([128, 512], F32, tag="pp")
                        for t in range(CT):
                            nc.tensor.matmul(
                                pp[:, :],
                                lhsT=w_outT_sb[t][:, 128 * m:128 * (m + 1)],
                                rhs=attn_sb[t][:, 512 * n:512 * (n + 1)],
                                start=(t == 0), stop=(t == CT - 1))
                        o_sb = small.tile([128, 512], F32, tag="osb", bufs=4)
                        nc.vector.scalar_tensor_tensor(
                            out=o_sb, in0=pp[:, :],
                            scalar=b_out_sb[:, m:m + 1],
                            in1=x_sb[m][:, 512 * n:512 * (n + 1)],
                            op0=ADD, op1=ADD)
                        nc.sync.dma_start(
                            out=out_ext[128 * m:128 * (m + 1),
                                        512 * n:512 * (n + 1)],
                            in_=o_sb)
    return nc


def _install_ntff_hook():
    """The agent image's antenv lacks axon_hooks; synthesize it so
    run_bass_kernel_spmd(trace=True) can reach the NTFF profiler."""
    import types
    if "antenv.axon_hooks" in sys.modules:
        return
    mod = types.ModuleType("antenv.axon_hooks")
    mod._hook = None

    def set_axon_ntff_profile_hook(hook):
        mod._hook = hook

    def get_axon_ntff_profile_hook():
        return mod._hook

    mod.set_axon_ntff_profile_hook = set_axon_ntff_profile_hook
    mod.get_axon_ntff_profile_hook = get_axon_ntff_profile_hook
    sys.modules["antenv.axon_hooks"] = mod
    try:
        from trn_agent_boot.trn_boot import _ntff_profile_via_ctypes
        hook = _ntff_profile_via_ctypes("/opt/axon/libaxon_pjrt.so")
        if hook is not None:
            set_axon_ntff_profile_hook(hook)
    except Exception as e:  # degrade to no tracing
        print("ntff hook setup failed:", e)


_COMPILED = None


def _get_compiled():
    global _COMPILED
    if _COMPILED is None:
        nc = build_graph()
        nc.compile()
        _COMPILED = nc
    return _COMPILED


def _make_consts():
    # within any 128-channel tile, partition p belongs to local group p//16
    sel = np.zeros((128, GPT), dtype=np.float32)
    selT = np.zeros((GPT, 128), dtype=np.float32)
    for p in range(128):
        sel[p, p // GS] = 1.0
        selT[p // GS, p] = 1.0
    return sel, selT


def _pm(v, cols):
    """[cols*128] vector -> partition-major [128, cols]."""
    return np.ascontiguousarray(v.reshape(cols, 128).T)


def kernel(x, gamma, beta, w_in, b_in, w_out, b_out, _trace=False):
    x = np.asarray(x, dtype=np.float32)
    gamma = np.asarray(gamma, dtype=np.float32)
    beta = np.asarray(beta, dtype=np.float32)
    w_in = np.asarray(w_in, dtype=np.float32)
    b_in = np.asarray(b_in, dtype=np.float32)
    w_out = np.asarray(w_out, dtype=np.float32)
    b_out = np.asarray(b_out, dtype=np.float32)

    w_inT = np.ascontiguousarray(w_in.T).astype(ml_dtypes.bfloat16)
    w_outT = np.ascontiguousarray(w_out.T).astype(ml_dtypes.bfloat16)
    sel, selT = _make_consts()
    # fold v-bias through proj_out: softmax rows sum to 1, so the attention
    # output is attn_raw + b_v exactly; w_out @ b_v + b_out replaces b_out.
    b_v = b_in[2 * HID:3 * HID]
    b_out_eff = b_out + w_out.astype(np.float64) @ b_v.astype(np.float64)
    b_out_eff = b_out_eff.astype(np.float32)
    common = {
        "w_inT": w_inT,
        "w_outT": w_outT,
        "b_in_pm": _pm(b_in[0:2 * HID], 8),
        "b_out_pm": _pm(b_out_eff, CT),
        "gamma_pm": _pm(gamma, CT),
        "beta_pm": _pm(beta, CT),
        "gn_sel": sel,
        "gn_selT": selT,
    }
    in_maps = []
    for b in range(B):
        m = dict(common)
        m["x"] = np.ascontiguousarray(x[b].reshape(C, HW)).astype(
            ml_dtypes.bfloat16)
        in_maps.append(m)

    if _trace:
        _install_ntff_hook()
    nc = _get_compiled()
    res = run_bass_kernel_spmd(nc, in_maps, core_ids=list(range(B)),
                               trace=_trace)
    out = np.stack([np.asarray(res.results[b]["out"]).reshape(C, H, W)
                    for b in range(B)])
    if _trace:
        return out, res
    return out


if __name__ == "__main__":
    rng = np.random.default_rng(0)
    inputs = {
        "x": rng.standard_normal((B, C, H, W), dtype=np.float32),
        "gamma": np.ones(C, dtype=np.float32),
        "beta": np.zeros(C, dtype=np.float32),
        "w_in": (rng.standard_normal((3 * HID, C), dtype=np.float32)
                 / np.sqrt(C)),
        "b_in": np.zeros(3 * HID, dtype=np.float32),
        "w_out": (rng.standard_normal((C, HID), dtype=np.float32)
                  / np.sqrt(HID)),
        "b_out": np.zeros(C, dtype=np.float32),
    }
    out = kernel(**inputs)
    print("kernel ran, out shape", out.shape)


# revision 15
# speedup vs baseline: 1.6139x; 1.0767x over previous
"""Trainium2 Bass kernel for nn_AttentionBlock (GroupNorm + 8-head attention
block on [8, 512, 32, 32], residual).

Sharding: pure data-parallel over batch B=8 across the 8 NeuronCores — one
batch element per core, weights replicated, zero collectives.

v3 schedule, built around keeping the Activation engine dense on the 64
[128,1024] exps (the kernel is ACT-bound):
  - GroupNorm is per-channel-tile (each 16-channel group lives inside one
    128-channel tile, so no cross-tile combine): h[t] is ready ~3us after
    x[t] lands, and proj_in starts immediately.
  - proj_in (q/k per head-pair) is interleaved with attention: exp of pair 0
    starts right after qk0+logits0-p0 instead of after all of proj_in.
  - out2 (v @ eT) matmuls of pair hp ride along with the logits matmuls of
    pair hp+1; pair 3's out2 p-steps chase its own exps.
  - softmax denominators: po row 64 (from the vT ones-column) is evicted
    together with attn_u in one [65,1024] bf16 copy; the denom rows take a
    transposing DMA round trip to a [128,2,8] layout where one
    reciprocal_approx_fast costs ~0.2us (DVE time is free-size-based, so
    the single-partition RECIPROCAL of the baseline was 6.5us/head);
    reciprocals return via the inverse DMA and a stride-0 broadcast.
    (reciprocal_approx_fast reads garbage from PSUM — SBUF source only.)
  - b_v is folded into b_out on the host (softmax rows sum to 1, so
    out = w_out@attn_raw + (w_out@b_v + b_out) exactly).
  - GroupNorm's h = a*x+d runs on DVE (tensor_scalar mult+add) to keep ACT
    free for exp; only the vT ones-column is memset.
"""
import sys

sys.path.insert(0, "/opt/trn_rl_repo")

import numpy as np
import ml_dtypes

import concourse.bass as bass
import concourse.bacc as bacc
import concourse.tile as tile
from concourse import mybir
from concourse.bass_utils import run_bass_kernel_spmd

F32 = mybir.dt.float32
BF16 = mybir.dt.bfloat16
ADD = mybir.AluOpType.add
MULT = mybir.AluOpType.mult

B, C, H, W = 8, 512, 32, 32
HW = H * W       # 1024
NG = 32          # groups
GS = C // NG     # 16 channels per group
NH = 8           # heads
HD = 64          # head dim
HID = NH * HD    # 512
NP = NH // 2     # 4 head pairs
EPS = 1e-6
SCALE = 1.0 / float(np.sqrt(HD))  # 0.125
CT = C // 128    # 4 channel partition-tiles
PT = HW // 128   # 8 pixel partition-tiles
GPT = NG // CT   # 8 groups per channel-tile
GN_INV = 1.0 / (GS * HW)          # 1/16384


def build_graph():
    nc = bacc.Bacc("TRN2", num_devices=8)

    x_ext = nc.declare_dram_parameter("x", [C, HW], BF16, isOutput=False)
    w_inT_ext = nc.declare_dram_parameter("w_inT", [C, 3 * HID], BF16, isOutput=False)
    w_outT_ext = nc.declare_dram_parameter("w_outT", [HID, C], BF16, isOutput=False)
    # packed [128, 28] consts: 0:4 gamma, 4:8 beta, 8:16 b_in(q,k),
    # 16:20 b_out_eff, 20:28 gn_sel
    cpack_ext = nc.declare_dram_parameter("cpack", [128, 28], F32, isOutput=False)
    selT_ext = nc.declare_dram_parameter("gn_selT", [GPT, 128], F32, isOutput=False)
    out_ext = nc.declare_dram_parameter("out", [C, HW], F32, isOutput=True)

    rden_dram = nc.dram_tensor("rden_scratch", [NH, HW], F32)

    with tile.TileContext(nc) as tc:
        with (
            tc.tile_pool(name="const", bufs=1) as const,
            tc.tile_pool(name="big", bufs=1) as big,
            tc.tile_pool(name="eT", bufs=1) as eTp,
            tc.tile_pool(name="small", bufs=2) as small,
        ):
            # ---------- loads: x first (GN critical path), then consts,
            # w_inT (needed at first qk matmul), w_outT last ----------
            x_sb = [big.tile([128, HW], BF16, tag=f"x{t}", name=f"x{t}")
                    for t in range(CT)]
            # gpsimd and sync each issue DMA descriptors at ~0.6us per
            # instruction, so loads are few and split across both engines.
            for t in range(CT):
                nc.gpsimd.dma_start(out=x_sb[t],
                                    in_=x_ext[128 * t:128 * (t + 1), :])
            cpack_sb = const.tile([128, 28], F32)
            nc.gpsimd.dma_start(out=cpack_sb, in_=cpack_ext[:, :])
            selT_sb = const.tile([GPT, 128], F32)
            nc.gpsimd.dma_start(out=selT_sb, in_=selT_ext[:, :])
            gamma_sb = cpack_sb[:, 0:4]
            beta_sb = cpack_sb[:, 4:8]
            b_in_sb = cpack_sb[:, 8:16]
            b_out_sb = cpack_sb[:, 16:20]
            sel_sb = cpack_sb[:, 20:28]
            w_inT_sb = [big.tile([128, 3 * HID], BF16, tag=f"wi{t}", name=f"wi{t}")
                        for t in range(CT)]
            for t in range(CT):
                for blk in range(2):  # q cols then k cols, per tile
                    nc.sync.dma_start(
                        out=w_inT_sb[t][:, HID * blk:HID * (blk + 1)],
                        in_=w_inT_ext[128 * t:128 * (t + 1),
                                      HID * blk:HID * (blk + 1)])
            for t in range(CT):
                nc.sync.dma_start(
                    out=w_inT_sb[t][:, 2 * HID:3 * HID],
                    in_=w_inT_ext[128 * t:128 * (t + 1), 2 * HID:3 * HID])
            w_outT_sb = [big.tile([128, C], BF16, tag=f"wo{t}", name=f"wo{t}")
                         for t in range(CT)]
            for t in range(CT):
                nc.sync.dma_start(out=w_outT_sb[t],
                                  in_=w_outT_ext[128 * t:128 * (t + 1), :])

            # ---------- groupnorm (per channel-tile) + qk0, interleaved ----------
            h_sb = [big.tile([128, HW], BF16, tag=f"h{t}", name=f"h{t}")
                    for t in range(CT)]
            # ---------- fused proj_in + attention ----------
            q_sb = [big.tile([128, HW], BF16, tag=f"q{m}", name=f"q{m}")
                    for m in range(NP)]
            k_sb = [big.tile([128, HW], BF16, tag=f"k{m}", name=f"k{m}")
                    for m in range(NP)]
            vT_sb = [big.tile([128, NH, HD + 1], BF16, tag=f"vT{p}",
                              name=f"vT{p}") for p in range(PT)]
            attn_sb = [big.tile([128, HW], BF16, tag=f"at{i}", name=f"at{i}")
                       for i in range(NP)]
            po_tiles = {}    # hp -> [po_sub0, po_sub1]
            eT_all = {}      # hp -> [[eT tiles sub0], [sub1]]

            def qk_burst(hp, which):
                """All 8 accumulation matmuls + evict for q or k of pair hp,
                emitted as one slot so the pbig ring is never held open."""
                dest, off, bc = ((q_sb, 0, hp) if which == "q"
                                 else (k_sb, HID, 4 + hp))
                pp = pbig.tile([128, HW], F32, tag="pb",
                               name=f"{which}_acc{hp}")
                for n in range(2):
                    for t in range(CT):
                        nc.tensor.matmul(
                            pp[:, 512 * n:512 * (n + 1)],
                            lhsT=w_inT_sb[t][:, off + 128 * hp:
                                             off + 128 * (hp + 1)],
                            rhs=h_sb[t][:, 512 * n:512 * (n + 1)],
                            start=(t == 0), stop=(t == CT - 1))
                nc.vector.tensor_scalar(
                    out=dest[hp], in0=pp[:, :],
                    scalar1=b_in_sb[:, bc:bc + 1], scalar2=None, op0=ADD)

            def v_burst(j):
                """v tiles 2j, 2j+1 (pv pool, own PSUM banks)."""
                for p in (2 * j, 2 * j + 1):
                    pp = pv.tile([128, 512], F32, tag="pv")
                    for t in range(CT):
                        nc.tensor.matmul(
                            pp[:, :],
                            lhsT=h_sb[t][:, 128 * p:128 * (p + 1)],
                            rhs=w_inT_sb[t][:, 2 * HID:3 * HID],
                            start=(t == 0), stop=(t == CT - 1))
                    nc.vector.tensor_copy(
                        out=vT_sb[p][:, :, 0:HD],
                        in_=pp[:, :].rearrange("a (nh c) -> a nh c", nh=NH))

            def out2_step(hp, p):
                """One accumulation p-step of pair hp's out2 (both heads)."""
                if p == 0:
                    po_tiles[hp] = [
                        pop.tile([HD + 1, HW], F32, tag="po",
                                 name=f"po{2 * hp + s}") for s in range(2)]
                eTs = eT_all[hp]
                for sub in range(2):
                    head = 2 * hp + sub
                    po_t = po_tiles[hp][sub]
                    for n in range(2):
                        nc.tensor.matmul(
                            po_t[:, 512 * n:512 * (n + 1)],
                            lhsT=vT_sb[p][:, head, :],
                            rhs=eTs[sub][p][:, 512 * n:512 * (n + 1)],
                            start=(p == 0), stop=(p == PT - 1))

            def emit_logits_exp(hp, out2_of=None, fillers=None):
                eTs = [[eTp.tile([128, HW], BF16, bufs=2, tag=f"eT{sub}_{p}",
                                 name=f"eT{hp}_{sub}_{p}") for p in range(PT)]
                       for sub in range(2)]
                eT_all[hp] = eTs
                for p in range(PT):
                    pls = []
                    for sub in range(2):
                        lo = 64 * sub
                        pl = pbig.tile([128, HW], F32, tag="pb",
                                       name=f"pl{hp}_{sub}_{p}")
                        for n in range(2):
                            nc.tensor.matmul(
                                pl[:, 512 * n:512 * (n + 1)],
                                lhsT=k_sb[hp][lo:lo + 64, 128 * p:128 * (p + 1)],
                                rhs=q_sb[hp][lo:lo + 64, 512 * n:512 * (n + 1)],
                                start=True, stop=True)
                        pls.append(pl)
                    for sub in range(2):
                        nc.scalar.activation(
                            out=eTs[sub][p], in_=pls[sub][:, :],
                            func=mybir.ActivationFunctionType.Exp,
                            scale=SCALE)
                    if out2_of is not None:
                        out2_step(out2_of, p)
                    for f in (fillers or {}).get(p, []):
                        f()

            def finish_pair(hp):
                """Denominator reciprocal + normalize, reading po in place.

                reciprocal_approx_fast misreads PSUM, so the denom row takes
                one [1,1024] DVE hop to SBUF first. The multiply reads po
                directly (PSUM f32 x rb f32), so po stays live until here —
                safe, its pool buffer isn't needed again for ~a pair-window.
                """
                eT_all.pop(hp)
                pos = po_tiles.pop(hp)
                au_s = []
                den2 = small.tile([2, HW], F32, tag="den2", bufs=2,
                                  name=f"den2_{hp}")
                rr2 = small.tile([2, HW], F32, tag="rr2", bufs=2,
                                 name=f"rr2_{hp}")
                for sub in range(2):
                    head = 2 * hp + sub
                    au = small.tile([HD, HW], BF16, tag="attnu", bufs=4,
                                    name=f"attnu{head}")
                    nc.vector.tensor_copy(out=au, in_=pos[sub][0:HD, :])
                    au_s.append(au)
                    rrow = small.tile([HD + 1, HW], F32, tag="rrow",
                                      bufs=2, name=f"rrow{head}")
                    nc.vector.tensor_copy(out=rrow[HD:HD + 1, :],
                                          in_=pos[sub][HD:HD + 1, :])
                    # reciprocal_approx_fast is wrong at partition base != 0:
                    # hop the row down to partition `sub` via SBUF->SBUF DMA.
                    nc.sync.dma_start(out=den2[sub:sub + 1, :],
                                      in_=rrow[HD:HD + 1, :])
                nc.vector.reciprocal_approx_fast(out=rr2, in_=den2)
                nc.sync.dma_start(out=rden_dram[2 * hp:2 * hp + 2, :],
                                  in_=rr2)
                for sub in range(2):
                    head = 2 * hp + sub
                    rb = small.tile([HD, HW], F32, tag="rb", bufs=2,
                                    name=f"rb{head}")
                    bcast_ap = bass.AP(
                        tensor=rden_dram[:, :].tensor,
                        offset=head * HW,
                        ap=[[0, HD], [1, HW]])
                    nc.sync.dma_start(out=rb, in_=bcast_ap)
                    if sub == 0:
                        nc.vector.tensor_mul(attn_sb[hp][0:HD, :],
                                             au_s[sub][:, :], rb[:, :])
                    else:
                        tmp2 = small.tile([HD, HW], BF16, tag="atmp2",
                                          bufs=2, name=f"atmp2{head}")
                        nc.vector.tensor_mul(tmp2[:, :], au_s[sub][:, :],
                                             rb[:, :])
                        nc.sync.dma_start(out=attn_sb[hp][HD:128, :],
                                          in_=tmp2)

            with tc.tile_pool(name="pbig", bufs=2, space="PSUM") as pbig:
                # GN per tile + qk0 accumulation steps ride along so the
                # first exp fires as soon as x3 lands.
                ppq0 = pbig.tile([128, HW], F32, tag="pb", name="q_acc0")
                ppk0 = pbig.tile([128, HW], F32, tag="pb", name="k_acc0")
                with tc.tile_pool(name="ps_gn", bufs=2, space="PSUM") as ps_gn:
                    eps_sb = small.tile([GPT, 1], F32, tag="eps_c", bufs=1)
                    nc.vector.memset(eps_sb, float(EPS))
                    sq_scratch = small.tile([128, HW], F32, tag="sqs", bufs=1)
                    for t in range(CT):
                        st = small.tile([128, 2], F32, tag=f"st{t}", bufs=1,
                                        name=f"st{t}")
                        nc.vector.reduce_sum(st[:, 0:1], x_sb[t][:, :],
                                             axis=mybir.AxisListType.X)
                        # sum of squares on DVE: (x bypass) * x, accum free
                        nc.vector.scalar_tensor_tensor(
                            out=sq_scratch, in0=x_sb[t][:, :], scalar=1.0,
                            in1=x_sb[t][:, :],
                            op0=mybir.AluOpType.bypass, op1=MULT,
                            accum_out=st[:, 1:2])
                        gpsum = ps_gn.tile([GPT, 2], F32, tag="gps")
                        nc.tensor.matmul(gpsum[:, :], lhsT=sel_sb[:, :],
                                         rhs=st[:, :], start=True, stop=True)
                        # grp cols: 0 rstd, 1 mean*rstd, 2 mean, 3 E[x^2]
                        grp = small.tile([GPT, 4], F32, tag="grp", bufs=2,
                                         name=f"grp{t}")
                        nc.vector.tensor_scalar_mul(grp[:, 2:4],
                                                    gpsum[:, 0:2], GN_INV)
                        nc.vector.tensor_mul(grp[:, 0:1], grp[:, 2:3],
                                             grp[:, 2:3])
                        nc.vector.tensor_sub(grp[:, 0:1], grp[:, 3:4],
                                             grp[:, 0:1])
                        nc.scalar.activation(
                            out=grp[:, 0:1], in_=grp[:, 0:1],
                            func=mybir.ActivationFunctionType.Sqrt,
                            bias=eps_sb[:, :], scale=1.0)
                        nc.vector.reciprocal(out=grp[:, 0:1], in_=grp[:, 0:1])
                        nc.vector.tensor_mul(grp[:, 1:2], grp[:, 2:3],
                                             grp[:, 0:1])
                        epsum = ps_gn.tile([128, 2], F32, tag="eps")
                        nc.tensor.matmul(epsum[:, :], lhsT=selT_sb[:, :],
                                         rhs=grp[:, 0:2], start=True,
                                         stop=True)
                        ga = small.tile([128, 1], F32, tag=f"ga{t}", bufs=1,
                                        name=f"ga{t}")
                        gd = small.tile([128, 1], F32, tag=f"gd{t}", bufs=1,
                                        name=f"gd{t}")
                        nc.vector.tensor_mul(ga[:, :], gamma_sb[:, t:t + 1],
                                             epsum[:, 0:1])
                        nc.vector.tensor_mul(gd[:, :], gamma_sb[:, t:t + 1],
                                             epsum[:, 1:2])
                        nc.vector.tensor_sub(gd[:, :], beta_sb[:, t:t + 1],
                                             gd[:, :])
                        nc.vector.tensor_scalar(
                            out=h_sb[t], in0=x_sb[t][:, :],
                            scalar1=ga[:, :], scalar2=gd[:, :],
                            op0=MULT, op1=ADD)
                        for n in range(2):
                            nc.tensor.matmul(
                                ppq0[:, 512 * n:512 * (n + 1)],
                                lhsT=w_inT_sb[t][:, 0:128],
                                rhs=h_sb[t][:, 512 * n:512 * (n + 1)],
                                start=(t == 0), stop=(t == CT - 1))
                            nc.tensor.matmul(
                                ppk0[:, 512 * n:512 * (n + 1)],
                                lhsT=w_inT_sb[t][:, HID:HID + 128],
                                rhs=h_sb[t][:, 512 * n:512 * (n + 1)],
                                start=(t == 0), stop=(t == CT - 1))
                nc.vector.tensor_scalar(
                    out=q_sb[0], in0=ppq0[:, :],
                    scalar1=b_in_sb[:, 0:1], scalar2=None, op0=ADD)
                nc.vector.tensor_scalar(
                    out=k_sb[0], in0=ppk0[:, :],
                    scalar1=b_in_sb[:, 4:5], scalar2=None, op0=ADD)
                for p in range(PT):
                    nc.vector.memset(vT_sb[p][:, :, HD:HD + 1], 1.0)
                with tc.tile_pool(name="pv", bufs=2, space="PSUM") as pv:
                    emit_logits_exp(0, fillers={
                        1: [lambda: qk_burst(1, "q")],
                        3: [lambda: v_burst(0)],
                        4: [lambda: v_burst(1)],
                        5: [lambda: qk_burst(1, "k")],
                        6: [lambda: v_burst(2)],
                        7: [lambda: v_burst(3)],
                    })
                with tc.tile_pool(name="po", bufs=2, space="PSUM") as pop:
                    emit_logits_exp(1, out2_of=0, fillers={
                        2: [lambda: qk_burst(2, "q")],
                        5: [lambda: qk_burst(2, "k")],
                    })
                    finish_pair(0)
                    emit_logits_exp(2, out2_of=1, fillers={
                        2: [lambda: qk_burst(3, "q")],
                        5: [lambda: qk_burst(3, "k")],
                    })
                    finish_pair(1)
                    emit_logits_exp(3, out2_of=2)
                    finish_pair(2)
                    for p in range(PT):
                        out2_step(3, p)
                    finish_pair(3)

            # ---------- proj_out + bias + residual ----------
            with tc.tile_pool(name="ps_pout", bufs=4, space="PSUM") as ps_pout:
                for m in range(CT):
                    for n in range(2):
                        pp = ps_pout.tile([128, 512], F32, tag="pp")
                        for t in range(CT):
                            nc.tensor.matmul(
                                pp[:, :],
                                lhsT=w_outT_sb[t][:, 128 * m:128 * (m + 1)],
                                rhs=attn_sb[t][:, 512 * n:512 * (n + 1)],
                                start=(t == 0), stop=(t == CT - 1))
                        o_sb = small.tile([128, 512], F32, tag="osb", bufs=4)
                        nc.vector.scalar_tensor_tensor(
                            out=o_sb, in0=pp[:, :],
                            scalar=b_out_sb[:, m:m + 1],
                            in1=x_sb[m][:, 512 * n:512 * (n + 1)],
                            op0=ADD, op1=ADD)
                        nc.sync.dma_start(
                            out=out_ext[128 * m:128 * (m + 1),
                                        512 * n:512 * (n + 1)],
                            in_=o_sb)
    return nc


def _install_ntff_hook():
    """The agent image's antenv lacks axon_hooks; synthesize it so
    run_bass_kernel_spmd(trace=True) can reach the NTFF profiler."""
    import types
    if "antenv.axon_hooks" in sys.modules:
        return
    mod = types.ModuleType("antenv.axon_hooks")
    mod._hook = None

    def set_axon_ntff_profile_hook(hook):
        mod._hook = hook

    def get_axon_ntff_profile_hook():
        return mod._hook

    mod.set_axon_ntff_profile_hook = set_axon_ntff_profile_hook
    mod.get_axon_ntff_profile_hook = get_axon_ntff_profile_hook
    sys.modules["antenv.axon_hooks"] = mod
    try:
        from trn_agent_boot.trn_boot import _ntff_profile_via_ctypes
        hook = _ntff_profile_via_ctypes("/opt/axon/libaxon_pjrt.so")
        if hook is not None:
            set_axon_ntff_profile_hook(hook)
    except Exception as e:  # degrade to no tracing
        print("ntff hook setup failed:", e)


_COMPILED = None


def _get_compiled():
    global _COMPILED
    if _COMPILED is None:
        nc = build_graph()
        nc.compile()
        _COMPILED = nc
    return _COMPILED


def _make_consts():
    # within any 128-channel tile, partition p belongs to local group p//16
    sel = np.zeros((128, GPT), dtype=np.float32)
    selT = np.zeros((GPT, 128), dtype=np.float32)
    for p in range(128):
        sel[p, p // GS] = 1.0
        selT[p // GS, p] = 1.0
    return sel, selT


def _pm(v, cols):
    """[cols*128] vector -> partition-major [128, cols]."""
    return np.ascontiguousarray(v.reshape(cols, 128).T)


def kernel(x, gamma, beta, w_in, b_in, w_out, b_out, _trace=False):
    x = np.asarray(x, dtype=np.float32)
    gamma = np.asarray(gamma, dtype=np.float32)
    beta = np.asarray(beta, dtype=np.float32)
    w_in = np.asarray(w_in, dtype=np.float32)
    b_in = np.asarray(b_in, dtype=np.float32)
    w_out = np.asarray(w_out, dtype=np.float32)
    b_out = np.asarray(b_out, dtype=np.float32)

    w_inT = np.ascontiguousarray(w_in.T).astype(ml_dtypes.bfloat16)
    w_outT = np.ascontiguousarray(w_out.T).astype(ml_dtypes.bfloat16)
    sel, selT = _make_consts()
    # fold v-bias through proj_out: softmax rows sum to 1, so the attention
    # output is attn_raw + b_v exactly; w_out @ b_v + b_out replaces b_out.
    b_v = b_in[2 * HID:3 * HID]
    b_out_eff = b_out + w_out.astype(np.float64) @ b_v.astype(np.float64)
    b_out_eff = b_out_eff.astype(np.float32)
    cpack = np.zeros((128, 28), dtype=np.float32)
    cpack[:, 0:4] = _pm(gamma, CT)
    cpack[:, 4:8] = _pm(beta, CT)
    cpack[:, 8:16] = _pm(b_in[0:2 * HID], 8)
    cpack[:, 16:20] = _pm(b_out_eff, CT)
    cpack[:, 20:28] = sel
    common = {
        "w_inT": w_inT,
        "w_outT": w_outT,
        "cpack": cpack,
        "gn_selT": selT,
    }
    in_maps = []
    for b in range(B):
        m = dict(common)
        m["x"] = np.ascontiguousarray(x[b].reshape(C, HW)).astype(
            ml_dtypes.bfloat16)
        in_maps.append(m)

    if _trace:
        _install_ntff_hook()
    nc = _get_compiled()
    res = run_bass_kernel_spmd(nc, in_maps, core_ids=list(range(B)),
                               trace=_trace)
    out = np.stack([np.asarray(res.results[b]["out"]).reshape(C, H, W)
                    for b in range(B)])
    if _trace:
        return out, res
    return out


if __name__ == "__main__":
    rng = np.random.default_rng(0)
    inputs = {
        "x": rng.standard_normal((B, C, H, W), dtype=np.float32),
        "gamma": np.ones(C, dtype=np.float32),
        "beta": np.zeros(C, dtype=np.float32),
        "w_in": (rng.standard_normal((3 * HID, C), dtype=np.float32)
                 / np.sqrt(C)),
        "b_in": np.zeros(3 * HID, dtype=np.float32),
        "w_out": (rng.standard_normal((C, HID), dtype=np.float32)
                  / np.sqrt(HID)),
        "b_out": np.zeros(C, dtype=np.float32),
    }
    out = kernel(**inputs)
    print("kernel ran, out shape", out.shape)


# revision 16
# speedup vs baseline: 1.6172x; 1.0021x over previous
"""Trainium2 Bass kernel for nn_AttentionBlock (GroupNorm + 8-head attention
block on [8, 512, 32, 32], residual).

Sharding: pure data-parallel over batch B=8 across the 8 NeuronCores — one
batch element per core, weights replicated, zero collectives.

v3 schedule, built around keeping the Activation engine dense on the 64
[128,1024] exps (the kernel is ACT-bound):
  - GroupNorm is per-channel-tile (each 16-channel group lives inside one
    128-channel tile, so no cross-tile combine): h[t] is ready ~3us after
    x[t] lands, and proj_in starts immediately.
  - proj_in (q/k per head-pair) is interleaved with attention: exp of pair 0
    starts right after qk0+logits0-p0 instead of after all of proj_in.
  - out2 (v @ eT) matmuls of pair hp ride along with the logits matmuls of
    pair hp+1; pair 3's out2 p-steps chase its own exps.
  - softmax denominators: po row 64 (from the vT ones-column) is evicted
    together with attn_u in one [65,1024] bf16 copy; the denom rows take a
    transposing DMA round trip to a [128,2,8] layout where one
    reciprocal_approx_fast costs ~0.2us (DVE time is free-size-based, so
    the single-partition RECIPROCAL of the baseline was 6.5us/head);
    reciprocals return via the inverse DMA and a stride-0 broadcast.
    (reciprocal_approx_fast reads garbage from PSUM — SBUF source only.)
  - b_v is folded into b_out on the host (softmax rows sum to 1, so
    out = w_out@attn_raw + (w_out@b_v + b_out) exactly).
  - GroupNorm's h = a*x+d runs on DVE (tensor_scalar mult+add) to keep ACT
    free for exp; only the vT ones-column is memset.
"""
import sys

sys.path.insert(0, "/opt/trn_rl_repo")

import numpy as np
import ml_dtypes

import concourse.bass as bass
import concourse.bacc as bacc
import concourse.tile as tile
from concourse import mybir
from concourse.bass_utils import run_bass_kernel_spmd

F32 = mybir.dt.float32
BF16 = mybir.dt.bfloat16
ADD = mybir.AluOpType.add
MULT = mybir.AluOpType.mult

B, C, H, W = 8, 512, 32, 32
HW = H * W       # 1024
NG = 32          # groups
GS = C // NG     # 16 channels per group
NH = 8           # heads
HD = 64          # head dim
HID = NH * HD    # 512
NP = NH // 2     # 4 head pairs
EPS = 1e-6
SCALE = 1.0 / float(np.sqrt(HD))  # 0.125
CT = C // 128    # 4 channel partition-tiles
PT = HW // 128   # 8 pixel partition-tiles
GPT = NG // CT   # 8 groups per channel-tile
GN_INV = 1.0 / (GS * HW)          # 1/16384


def build_graph():
    nc = bacc.Bacc("TRN2", num_devices=8)

    x_ext = nc.declare_dram_parameter("x", [C, HW], BF16, isOutput=False)
    w_inT_ext = nc.declare_dram_parameter("w_inT", [C, 3 * HID], BF16, isOutput=False)
    w_outT_ext = nc.declare_dram_parameter("w_outT", [HID, C], BF16, isOutput=False)
    # packed [128, 28] consts: 0:4 gamma, 4:8 beta, 8:16 b_in(q,k),
    # 16:20 b_out_eff, 20:28 gn_sel
    cpack_ext = nc.declare_dram_parameter("cpack", [128, 28], F32, isOutput=False)
    selT_ext = nc.declare_dram_parameter("gn_selT", [GPT, 128], F32, isOutput=False)
    out_ext = nc.declare_dram_parameter("out", [C, HW], F32, isOutput=True)

    rden_dram = nc.dram_tensor("rden_scratch", [NH, HW], F32)

    with tile.TileContext(nc) as tc:
        with (
            tc.tile_pool(name="const", bufs=1) as const,
            tc.tile_pool(name="big", bufs=1) as big,
            tc.tile_pool(name="eT", bufs=1) as eTp,
            tc.tile_pool(name="small", bufs=2) as small,
        ):
            # ---------- loads: x first (GN critical path), then consts,
            # w_inT (needed at first qk matmul), w_outT last ----------
            x_sb = [big.tile([128, HW], BF16, tag=f"x{t}", name=f"x{t}")
                    for t in range(CT)]
            # gpsimd and sync each issue DMA descriptors at ~0.6us per
            # instruction, so loads are few and split across both engines.
            for t in (0, 2):
                nc.gpsimd.dma_start(out=x_sb[t],
                                    in_=x_ext[128 * t:128 * (t + 1), :])
            for t in (1, 3):
                nc.sync.dma_start(out=x_sb[t],
                                  in_=x_ext[128 * t:128 * (t + 1), :])
            cpack_sb = const.tile([128, 28], F32)
            nc.gpsimd.dma_start(out=cpack_sb, in_=cpack_ext[:, :])
            selT_sb = const.tile([GPT, 128], F32)
            nc.gpsimd.dma_start(out=selT_sb, in_=selT_ext[:, :])
            # remaining weight loads issue on gpsimd so sync stays free for
            # the attention-phase scratch DMAs
            gamma_sb = cpack_sb[:, 0:4]
            beta_sb = cpack_sb[:, 4:8]
            b_in_sb = cpack_sb[:, 8:16]
            b_out_sb = cpack_sb[:, 16:20]
            sel_sb = cpack_sb[:, 20:28]
            w_inT_sb = [big.tile([128, 3 * HID], BF16, tag=f"wi{t}", name=f"wi{t}")
                        for t in range(CT)]
            for t in range(CT):
                for blk in range(2):  # q cols then k cols, per tile
                    nc.gpsimd.dma_start(
                        out=w_inT_sb[t][:, HID * blk:HID * (blk + 1)],
                        in_=w_inT_ext[128 * t:128 * (t + 1),
                                      HID * blk:HID * (blk + 1)])
            for t in range(CT):
                nc.gpsimd.dma_start(
                    out=w_inT_sb[t][:, 2 * HID:3 * HID],
                    in_=w_inT_ext[128 * t:128 * (t + 1), 2 * HID:3 * HID])
            w_outT_sb = [big.tile([128, C], BF16, tag=f"wo{t}", name=f"wo{t}")
                         for t in range(CT)]
            for t in range(CT):
                nc.gpsimd.dma_start(out=w_outT_sb[t],
                                    in_=w_outT_ext[128 * t:128 * (t + 1), :])

            # ---------- groupnorm (per channel-tile) + qk0, interleaved ----------
            h_sb = [big.tile([128, HW], BF16, tag=f"h{t}", name=f"h{t}")
                    for t in range(CT)]
            # ---------- fused proj_in + attention ----------
            q_sb = [big.tile([128, HW], BF16, tag=f"q{m}", name=f"q{m}")
                    for m in range(NP)]
            k_sb = [big.tile([128, HW], BF16, tag=f"k{m}", name=f"k{m}")
                    for m in range(NP)]
            vT_sb = [big.tile([128, NH, HD + 1], BF16, tag=f"vT{p}",
                              name=f"vT{p}") for p in range(PT)]
            attn_sb = [big.tile([128, HW], BF16, tag=f"at{i}", name=f"at{i}")
                       for i in range(NP)]
            po_tiles = {}    # hp -> [po_sub0, po_sub1]
            eT_all = {}      # hp -> [[eT tiles sub0], [sub1]]

            def qk_burst(hp, which):
                """All 8 accumulation matmuls + evict for q or k of pair hp,
                emitted as one slot so the pbig ring is never held open."""
                dest, off, bc = ((q_sb, 0, hp) if which == "q"
                                 else (k_sb, HID, 4 + hp))
                pp = pbig.tile([128, HW], F32, tag="pb",
                               name=f"{which}_acc{hp}")
                for n in range(2):
                    for t in range(CT):
                        nc.tensor.matmul(
                            pp[:, 512 * n:512 * (n + 1)],
                            lhsT=w_inT_sb[t][:, off + 128 * hp:
                                             off + 128 * (hp + 1)],
                            rhs=h_sb[t][:, 512 * n:512 * (n + 1)],
                            start=(t == 0), stop=(t == CT - 1))
                nc.vector.tensor_scalar(
                    out=dest[hp], in0=pp[:, :],
                    scalar1=b_in_sb[:, bc:bc + 1], scalar2=None, op0=ADD)

            def v_one(p):
                """One v tile (pv pool, own PSUM banks)."""
                if True:
                    pp = pv.tile([128, 512], F32, tag="pv")
                    for t in range(CT):
                        nc.tensor.matmul(
                            pp[:, :],
                            lhsT=h_sb[t][:, 128 * p:128 * (p + 1)],
                            rhs=w_inT_sb[t][:, 2 * HID:3 * HID],
                            start=(t == 0), stop=(t == CT - 1))
                    nc.vector.tensor_copy(
                        out=vT_sb[p][:, :, 0:HD],
                        in_=pp[:, :].rearrange("a (nh c) -> a nh c", nh=NH))

            def out2_step(hp, p):
                """One accumulation p-step of pair hp's out2 (both heads)."""
                if p == 0:
                    po_tiles[hp] = [
                        pop.tile([HD + 1, HW], F32, tag="po",
                                 name=f"po{2 * hp + s}") for s in range(2)]
                eTs = eT_all[hp]
                for sub in range(2):
                    head = 2 * hp + sub
                    po_t = po_tiles[hp][sub]
                    for n in range(2):
                        nc.tensor.matmul(
                            po_t[:, 512 * n:512 * (n + 1)],
                            lhsT=vT_sb[p][:, head, :],
                            rhs=eTs[sub][p][:, 512 * n:512 * (n + 1)],
                            start=(p == 0), stop=(p == PT - 1))

            def emit_logits_exp(hp, out2_of=None, fillers=None):
                eTs = [[eTp.tile([128, HW], BF16, bufs=2, tag=f"eT{sub}_{p}",
                                 name=f"eT{hp}_{sub}_{p}") for p in range(PT)]
                       for sub in range(2)]
                eT_all[hp] = eTs
                for p in range(PT):
                    pls = []
                    for sub in range(2):
                        lo = 64 * sub
                        pl = pbig.tile([128, HW], F32, tag="pb",
                                       name=f"pl{hp}_{sub}_{p}")
                        for n in range(2):
                            nc.tensor.matmul(
                                pl[:, 512 * n:512 * (n + 1)],
                                lhsT=k_sb[hp][lo:lo + 64, 128 * p:128 * (p + 1)],
                                rhs=q_sb[hp][lo:lo + 64, 512 * n:512 * (n + 1)],
                                start=True, stop=True)
                        pls.append(pl)
                    for sub in range(2):
                        nc.scalar.activation(
                            out=eTs[sub][p], in_=pls[sub][:, :],
                            func=mybir.ActivationFunctionType.Exp,
                            scale=SCALE)
                    if out2_of is not None:
                        out2_step(out2_of, p)
                    for f in (fillers or {}).get(p, []):
                        f()

            pair_state = {}

            def finish_den(hp):
                """Start the denominator chain ASAP: den row hop to
                partition base 0 (reciprocal_approx_fast is wrong at
                nonzero base), approx, write reciprocals to DRAM. attn_u is
                evicted here too so the po buffers free quickly."""
                eT_all.pop(hp)
                pos = po_tiles.pop(hp)
                den2 = small.tile([2, HW], F32, tag="den2", bufs=2,
                                  name=f"den2_{hp}")
                rr2 = small.tile([2, HW], F32, tag="rr2", bufs=2,
                                 name=f"rr2_{hp}")
                au_s = []
                for sub in range(2):
                    head = 2 * hp + sub
                    rrow = small.tile([HD + 1, HW], F32, tag="rrow",
                                      bufs=2, name=f"rrow{head}")
                    nc.vector.tensor_copy(out=rrow[HD:HD + 1, :],
                                          in_=pos[sub][HD:HD + 1, :])
                    nc.sync.dma_start(out=den2[sub:sub + 1, :],
                                      in_=rrow[HD:HD + 1, :])
                    au = small.tile([HD, HW], BF16, tag="attnu", bufs=4,
                                    name=f"attnu{head}")
                    nc.vector.tensor_copy(out=au, in_=pos[sub][0:HD, :])
                    au_s.append(au)
                nc.vector.reciprocal_approx_fast(out=rr2, in_=den2)
                nc.sync.dma_start(out=rden_dram[2 * hp:2 * hp + 2, :],
                                  in_=rr2)
                pair_state[hp] = au_s

            def finish_mul(hp):
                """Normalize: emitted a pair-window after finish_den so the
                DMA round trip never head-of-line blocks the DVE queue."""
                au_s = pair_state.pop(hp)
                for sub in range(2):
                    head = 2 * hp + sub
                    rb = small.tile([HD, HW], F32, tag="rb", bufs=2,
                                    name=f"rb{head}")
                    bcast_ap = bass.AP(
                        tensor=rden_dram[:, :].tensor,
                        offset=head * HW,
                        ap=[[0, HD], [1, HW]])
                    nc.sync.dma_start(out=rb, in_=bcast_ap)
                    if sub == 0:
                        nc.vector.tensor_mul(attn_sb[hp][0:HD, :],
                                             au_s[sub][:, :], rb[:, :])
                    else:
                        tmp2 = small.tile([HD, HW], BF16, tag="atmp2",
                                          bufs=2, name=f"atmp2{head}")
                        nc.vector.tensor_mul(tmp2[:, :], au_s[sub][:, :],
                                             rb[:, :])
                        nc.sync.dma_start(out=attn_sb[hp][HD:128, :],
                                          in_=tmp2)

            with tc.tile_pool(name="pbig", bufs=2, space="PSUM") as pbig:
                # GN per tile + qk0 accumulation steps ride along so the
                # first exp fires as soon as x3 lands.
                ppq0 = pbig.tile([128, HW], F32, tag="pb", name="q_acc0")
                ppk0 = pbig.tile([128, HW], F32, tag="pb", name="k_acc0")
                with tc.tile_pool(name="ps_gn", bufs=2, space="PSUM") as ps_gn:
                    eps_sb = small.tile([GPT, 1], F32, tag="eps_c", bufs=1)
                    nc.vector.memset(eps_sb, float(EPS))
                    sq_scratch = small.tile([128, HW], F32, tag="sqs", bufs=1)
                    for t in range(CT):
                        st = small.tile([128, 2], F32, tag=f"st{t}", bufs=1,
                                        name=f"st{t}")
                        nc.vector.reduce_sum(st[:, 0:1], x_sb[t][:, :],
                                             axis=mybir.AxisListType.X)
                        # sum of squares on DVE: (x bypass) * x, accum free
                        nc.vector.scalar_tensor_tensor(
                            out=sq_scratch, in0=x_sb[t][:, :], scalar=1.0,
                            in1=x_sb[t][:, :],
                            op0=mybir.AluOpType.bypass, op1=MULT,
                            accum_out=st[:, 1:2])
                        gpsum = ps_gn.tile([GPT, 2], F32, tag="gps")
                        nc.tensor.matmul(gpsum[:, :], lhsT=sel_sb[:, :],
                                         rhs=st[:, :], start=True, stop=True)
                        # grp cols: 0 rstd, 1 mean*rstd, 2 mean, 3 E[x^2]
                        grp = small.tile([GPT, 4], F32, tag="grp", bufs=2,
                                         name=f"grp{t}")
                        nc.vector.tensor_scalar_mul(grp[:, 2:4],
                                                    gpsum[:, 0:2], GN_INV)
                        nc.vector.tensor_mul(grp[:, 0:1], grp[:, 2:3],
                                             grp[:, 2:3])
                        nc.vector.tensor_sub(grp[:, 0:1], grp[:, 3:4],
                                             grp[:, 0:1])
                        nc.scalar.activation(
                            out=grp[:, 0:1], in_=grp[:, 0:1],
                            func=mybir.ActivationFunctionType.Sqrt,
                            bias=eps_sb[:, :], scale=1.0)
                        nc.vector.reciprocal(out=grp[:, 0:1], in_=grp[:, 0:1])
                        nc.vector.tensor_mul(grp[:, 1:2], grp[:, 2:3],
                                             grp[:, 0:1])
                        epsum = ps_gn.tile([128, 2], F32, tag="eps")
                        nc.tensor.matmul(epsum[:, :], lhsT=selT_sb[:, :],
                                         rhs=grp[:, 0:2], start=True,
                                         stop=True)
                        ga = small.tile([128, 1], F32, tag=f"ga{t}", bufs=1,
                                        name=f"ga{t}")
                        gd = small.tile([128, 1], F32, tag=f"gd{t}", bufs=1,
                                        name=f"gd{t}")
                        nc.vector.tensor_mul(ga[:, :], gamma_sb[:, t:t + 1],
                                             epsum[:, 0:1])
                        nc.vector.tensor_mul(gd[:, :], gamma_sb[:, t:t + 1],
                                             epsum[:, 1:2])
                        nc.vector.tensor_sub(gd[:, :], beta_sb[:, t:t + 1],
                                             gd[:, :])
                        nc.vector.tensor_scalar(
                            out=h_sb[t], in0=x_sb[t][:, :],
                            scalar1=ga[:, :], scalar2=gd[:, :],
                            op0=MULT, op1=ADD)
                        for n in range(2):
                            nc.tensor.matmul(
                                ppq0[:, 512 * n:512 * (n + 1)],
                                lhsT=w_inT_sb[t][:, 0:128],
                                rhs=h_sb[t][:, 512 * n:512 * (n + 1)],
                                start=(t == 0), stop=(t == CT - 1))
                            nc.tensor.matmul(
                                ppk0[:, 512 * n:512 * (n + 1)],
                                lhsT=w_inT_sb[t][:, HID:HID + 128],
                                rhs=h_sb[t][:, 512 * n:512 * (n + 1)],
                                start=(t == 0), stop=(t == CT - 1))
                nc.vector.tensor_scalar(
                    out=q_sb[0], in0=ppq0[:, :],
                    scalar1=b_in_sb[:, 0:1], scalar2=None, op0=ADD)
                nc.vector.tensor_scalar(
                    out=k_sb[0], in0=ppk0[:, :],
                    scalar1=b_in_sb[:, 4:5], scalar2=None, op0=ADD)
                for p in range(PT):
                    nc.vector.memset(vT_sb[p][:, :, HD:HD + 1], 1.0)
                with tc.tile_pool(name="pv", bufs=2, space="PSUM") as pv:
                    emit_logits_exp(0, fillers={
                        0: [lambda: v_one(0)],
                        1: [lambda: v_one(1)],
                        2: [lambda: qk_burst(1, "q")],
                        3: [lambda: v_one(2)],
                        4: [lambda: v_one(3)],
                        5: [lambda: qk_burst(1, "k")],
                        6: [lambda: v_one(4), lambda: v_one(5)],
                        7: [lambda: v_one(6), lambda: v_one(7)],
                    })
                with tc.tile_pool(name="po", bufs=2, space="PSUM") as pop:
                    emit_logits_exp(1, out2_of=0, fillers={
                        2: [lambda: qk_burst(2, "q")],
                        5: [lambda: qk_burst(2, "k")],
                    })
                    finish_den(0)
                    emit_logits_exp(2, out2_of=1, fillers={
                        2: [lambda: qk_burst(3, "q")],
                        5: [lambda: qk_burst(3, "k")],
                    })
                    finish_den(1)
                    finish_mul(0)
                    emit_logits_exp(3, out2_of=2)
                    finish_den(2)
                    finish_mul(1)
                    for p in range(PT):
                        out2_step(3, p)
                    finish_den(3)
                    finish_mul(2)
                    finish_mul(3)

            # ---------- proj_out + bias + residual ----------
            with tc.tile_pool(name="ps_pout", bufs=4, space="PSUM") as ps_pout:
                for m in range(CT):
                    for n in range(2):
                        pp = ps_pout.tile([128, 512], F32, tag="pp")
                        for t in range(CT):
                            nc.tensor.matmul(
                                pp[:, :],
                                lhsT=w_outT_sb[t][:, 128 * m:128 * (m + 1)],
                                rhs=attn_sb[t][:, 512 * n:512 * (n + 1)],
                                start=(t == 0), stop=(t == CT - 1))
                        o_sb = small.tile([128, 512], F32, tag="osb", bufs=4)
                        nc.vector.scalar_tensor_tensor(
                            out=o_sb, in0=pp[:, :],
                            scalar=b_out_sb[:, m:m + 1],
                            in1=x_sb[m][:, 512 * n:512 * (n + 1)],
                            op0=ADD, op1=ADD)
                        nc.sync.dma_start(
                            out=out_ext[128 * m:128 * (m + 1),
                                        512 * n:512 * (n + 1)],
                            in_=o_sb)
    return nc


def _install_ntff_hook():
    """The agent image's antenv lacks axon_hooks; synthesize it so
    run_bass_kernel_spmd(trace=True) can reach the NTFF profiler."""
    import types
    if "antenv.axon_hooks" in sys.modules:
        return
    mod = types.ModuleType("antenv.axon_hooks")
    mod._hook = None

    def set_axon_ntff_profile_hook(hook):
        mod._hook = hook

    def get_axon_ntff_profile_hook():
        return mod._hook

    mod.set_axon_ntff_profile_hook = set_axon_ntff_profile_hook
    mod.get_axon_ntff_profile_hook = get_axon_ntff_profile_hook
    sys.modules["antenv.axon_hooks"] = mod
    try:
        from trn_agent_boot.trn_boot import _ntff_profile_via_ctypes
        hook = _ntff_profile_via_ctypes("/opt/axon/libaxon_pjrt.so")
        if hook is not None:
            set_axon_ntff_profile_hook(hook)
    except Exception as e:  # degrade to no tracing
        print("ntff hook setup failed:", e)


_COMPILED = None


def _get_compiled():
    global _COMPILED
    if _COMPILED is None:
        nc = build_graph()
        nc.compile()
        _COMPILED = nc
    return _COMPILED


def _make_consts():
    # within any 128-channel tile, partition p belongs to local group p//16
    sel = np.zeros((128, GPT), dtype=np.float32)
    selT = np.zeros((GPT, 128), dtype=np.float32)
    for p in range(128):
        sel[p, p // GS] = 1.0
        selT[p // GS, p] = 1.0
    return sel, selT


def _pm(v, cols):
    """[cols*128] vector -> partition-major [128, cols]."""
    return np.ascontiguousarray(v.reshape(cols, 128).T)


def kernel(x, gamma, beta, w_in, b_in, w_out, b_out, _trace=False):
    x = np.asarray(x, dtype=np.float32)
    gamma = np.asarray(gamma, dtype=np.float32)
    beta = np.asarray(beta, dtype=np.float32)
    w_in = np.asarray(w_in, dtype=np.float32)
    b_in = np.asarray(b_in, dtype=np.float32)
    w_out = np.asarray(w_out, dtype=np.float32)
    b_out = np.asarray(b_out, dtype=np.float32)

    w_inT = np.ascontiguousarray(w_in.T).astype(ml_dtypes.bfloat16)
    w_outT = np.ascontiguousarray(w_out.T).astype(ml_dtypes.bfloat16)
    sel, selT = _make_consts()
    # fold v-bias through proj_out: softmax rows sum to 1, so the attention
    # output is attn_raw + b_v exactly; w_out @ b_v + b_out replaces b_out.
    b_v = b_in[2 * HID:3 * HID]
    b_out_eff = b_out + w_out.astype(np.float64) @ b_v.astype(np.float64)
    b_out_eff = b_out_eff.astype(np.float32)
    cpack = np.zeros((128, 28), dtype=np.float32)
    cpack[:, 0:4] = _pm(gamma, CT)
    cpack[:, 4:8] = _pm(beta, CT)
    cpack[:, 8:16] = _pm(b_in[0:2 * HID], 8)
    cpack[:, 16:20] = _pm(b_out_eff, CT)
    cpack[:, 20:28] = sel
    common = {
        "w_inT": w_inT,
        "w_outT": w_outT,
        "cpack": cpack,
        "gn_selT": selT,
    }
    in_maps = []
    for b in range(B):
        m = dict(common)
        m["x"] = np.ascontiguousarray(x[b].reshape(C, HW)).astype(
            ml_dtypes.bfloat16)
        in_maps.append(m)

    if _trace:
        _install_ntff_hook()
    nc = _get_compiled()
    res = run_bass_kernel_spmd(nc, in_maps, core_ids=list(range(B)),
                               trace=_trace)
    out = np.stack([np.asarray(res.results[b]["out"]).reshape(C, H, W)
                    for b in range(B)])
    if _trace:
        return out, res
    return out


if __name__ == "__main__":
    rng = np.random.default_rng(0)
    inputs = {
        "x": rng.standard_normal((B, C, H, W), dtype=np.float32),
        "gamma": np.ones(C, dtype=np.float32),
        "beta": np.zeros(C, dtype=np.float32),
        "w_in": (rng.standard_normal((3 * HID, C), dtype=np.float32)
                 / np.sqrt(C)),
        "b_in": np.zeros(3 * HID, dtype=np.float32),
        "w_out": (rng.standard_normal((C, HID), dtype=np.float32)
                  / np.sqrt(HID)),
        "b_out": np.zeros(C, dtype=np.float32),
    }
    out = kernel(**inputs)
    print("kernel ran, out shape", out.shape)
